# revision 1
# baseline (speedup 1.0000x reference)
"""Deformable-conv kernel for Trainium2: 8-core data-parallel over batch.

kernel(x, offset_w, offset_b, conv_w) -> [8, 128, 56, 56] float32.
Each NeuronCore processes one batch image:
  offset conv in true-F32 PE matmuls (the reference sampler is discontinuous
  at integer x-coords, so offsets need ~1e-7 accuracy to reproduce its
  floor/trunc decisions) -> pixel-partitioned offsets (PE transpose)
  -> index/bilinear-weight math (DVE) -> bf16 indirect-DMA gathers from a
  row-pair-interleaved padded map (interior pixels: one 512B descriptor per
  (pixel, tap) fetching the anti-diagonal [bot-left, top-right] corner pair,
  index shifted by the exact-integer-hit mask; edge tiles: one 1KB 4-corner
  descriptor) -> 2-term (interior) / 4-term (edge) blend (DVE) -> bf16 PE
  transpose -> 3x3/stride-3 conv as 9 accumulating bf16 matmuls (PSUM).
"""
import sys
for _p in ("/opt/trn_rl_repo", "/root/.axon_site/_ro/trn_rl_repo"):
    if _p not in sys.path:
        sys.path.append(_p)

from contextlib import ExitStack

import numpy as np
import ml_dtypes

import concourse.bass as bass
import concourse.bacc as bacc
import concourse.mybir as mybir
import concourse.tile as tile
from concourse.masks import make_identity
from concourse.bass_utils import run_bass_kernel_spmd
from concourse.bass_interp import get_hw_module

F32 = mybir.dt.float32
BF16 = mybir.dt.bfloat16
I32 = mybir.dt.int32
I16 = mybir.dt.int16
ALU = mybir.AluOpType
ACTF = mybir.ActivationFunctionType

DEBUG_FIX = False
USE_FIXUP = False
H = W = 56
HP = 58
NPIX = H * W          # 3136
NPAD = 3200           # padded pixel count (25 tiles of 128)
NTILE = 25
NTAP = 9
C = 128
OUT = 128
XR_ROWS = 3540 * 2    # interleaved row-pair map: entry e -> rows 2e, 2e+1
# edge tiles: pixel cols j<=2 or j>=52 live here (clip/trunc can fire in x)
EDGE_T = (0, 23, 24)
INT_T0, INT_T1 = 1, 23  # interior tiles [1, 23)


def build_kernel(nc):
    d = {
        "xcp": nc.dram_tensor("xcp", [C, HP * HP], F32, kind="ExternalInput").ap(),
        "xpm": nc.dram_tensor("xpm", [HP * HP, C], F32, kind="ExternalInput").ap(),
        "xr2": nc.dram_tensor("xr2", [XR_ROWS, C], BF16, kind="ExternalInput").ap(),
        "offw": nc.dram_tensor("offw", [C, NTAP * 18], F32, kind="ExternalInput").ap(),
        "offb": nc.dram_tensor("offb", [18, 1], F32, kind="ExternalInput").ap(),
        "convw": nc.dram_tensor("convw", [C, NTAP * OUT], BF16, kind="ExternalInput").ap(),
        "base": nc.dram_tensor("base", [C, NTILE * 18], F32, kind="ExternalInput").ap(),
        "tri": nc.dram_tensor("tri", [C, C], F32, kind="ExternalInput").ap(),
        "pixid1": nc.dram_tensor("pixid1", [C, NTILE], F32, kind="ExternalInput").ap(),
        "out": nc.dram_tensor("out", [OUT, NPAD], F32, kind="ExternalOutput").ap(),
    }
    if DEBUG_FIX:
        for nm, shp, dt in (("dbg_offTm", [C, NTILE * 18], F32),
                            ("dbg_offT2", [C, NTILE * 18], F32),
                            ("dbg_fl", [C, NTILE], F32),
                            ("dbg_rank", [C, NTILE], F32),
                            ("dbg_tb", [16, 8], F32),
                            ("dbg_delta", [C, 18], F32),
                            ("dbg_prec", [18, C], F32),
                            ("dbg_gk", [C, 3 * 3 * C], F32)):
            d[nm] = nc.dram_tensor(nm, shp, dt, kind="ExternalOutput").ap()
    with tile.TileContext(nc) as tc:
        emit(tc, d)
    return nc


def emit(tc, d):
    d_xcp, d_xpm, d_xr2 = d["xcp"], d["xpm"], d["xr2"]
    d_offw, d_offb, d_convw = d["offw"], d["offb"], d["convw"]
    d_base, d_tri, d_pixid1, d_out = d["base"], d["tri"], d["pixid1"], d["out"]
    nc = tc.nc
    F32R = mybir.dt.float32r
    ctx = ExitStack()
    with ctx:
        consts = ctx.enter_context(tc.tile_pool(name="consts", bufs=1))
        sb = ctx.enter_context(tc.tile_pool(name="sb", bufs=1))
        gpool = ctx.enter_context(tc.tile_pool(name="gpool", bufs=3))
        xpool = ctx.enter_context(tc.tile_pool(name="xpool", bufs=8))
        rpool = ctx.enter_context(tc.tile_pool(name="rpool", bufs=2))
        opool = ctx.enter_context(tc.tile_pool(name="opool", bufs=2))
        dpool = ctx.enter_context(tc.tile_pool(name="dpool", bufs=1, space="DRAM"))
        psBctx = ExitStack()
        psB = psBctx.enter_context(tc.tile_pool(name="psB", bufs=1, space="PSUM"))

        # ---- A: loads ----
        xcpr = None
        if USE_FIXUP:
            xcpr = consts.tile([C, HP * HP], F32R)
            nc.gpsimd.dma_start(xcpr[:], d_xcp[:])
        xcpf = consts.tile([C, HP * HP], F32)
        nc.sync.dma_start(xcpf[:], d_xcp[:])
        offw = consts.tile([C, NTAP * 18], F32)
        nc.sync.dma_start(offw[:], d_offw[:])
        offwr = None
        if USE_FIXUP:
            offwr = consts.tile([C, NTAP * 18], F32R)
            nc.gpsimd.dma_start(offwr[:], d_offw[:])
        convw = consts.tile([C, NTAP * OUT], BF16)
        nc.sync.dma_start(convw[:], d_convw[:])
        offb = consts.tile([18, 1], F32)
        nc.sync.dma_start(offb[:], d_offb[:])
        base = consts.tile([C, NTILE * 18], F32)
        nc.sync.dma_start(base[:], d_base[:])
        if USE_FIXUP:
            tri = consts.tile([C, C], F32)
            nc.sync.dma_start(tri[:], d_tri[:])
            pixid1 = consts.tile([C, NTILE], F32)
            nc.sync.dma_start(pixid1[:], d_pixid1[:])
        ident = consts.tile([C, C], F32)
        make_identity(nc, ident[:])
        identb = consts.tile([C, C], BF16)
        nc.vector.tensor_copy(out=identb[:], in_=ident[:])

        # ---- B: offset conv, fast F32R pass (fixed up below for pixels whose
        # x-offset lands near an integer, where the reference's trunc/floor
        # decisions are discontinuous) ----
        # col-major output pixels: chunk c covers j in [8c, 8c+8), all i.
        off_sb = sb.tile([18, NPAD], F32)
        xcp3 = (xcpr if USE_FIXUP else xcpf)[:].rearrange("p (y x) -> p y x", y=HP)
        pss = [psB.tile([18, 448], F32, tag=f"psB{ch}", name=f"psB{ch}")
               for ch in range(7)]
        for tap in range(NTAP):
            ky, kx = tap // 3, tap % 3
            for ch in range(7):
                rhs = xcp3[:, ky:ky + 56, kx + 8 * ch: kx + 8 * ch + 8] \
                    .transpose([0, 2, 1])
                lhsw = offwr if USE_FIXUP else offw
                nc.tensor.matmul(
                    pss[ch][:], lhsw[:, tap * 18:(tap + 1) * 18], rhs,
                    start=(tap == 0), stop=(tap == NTAP - 1))
        for ch in range(7):
            nc.scalar.activation(off_sb[:, 448 * ch:448 * (ch + 1)], pss[ch][:],
                                 ACTF.Identity, bias=offb[:, :1], scale=1.0)
        # pad pixels: 0.5 keeps them far from the near-integer flag band
        nc.vector.memset(off_sb[:, NPIX:], 0.5)

        # ---- C: transpose offsets to pixel-partitioned ----
        psBctx.close()
        psCctx = ExitStack()
        psC = psCctx.enter_context(tc.tile_pool(name="psC", bufs=2, space="PSUM"))
        offT = sb.tile([C, NTILE * 18], F32)
        for t in range(NTILE):
            pst = psC.tile([C, 18], F32, tag="psC")
            nc.tensor.transpose(pst[:], off_sb[:, t * C:(t + 1) * C],
                                ident[:18, :18])
            nc.scalar.activation(offT[:, t * 18:(t + 1) * 18], pst[:],
                                 ACTF.Copy)

        # ---- B2: precise fixup of near-integer x-offsets ----
        # Flag pixels with any x-offset within TH of an integer, zero their
        # x-offsets, round-trip offsets through DRAM (64-f32-padded pixel
        # rows), recompute flagged pixels' offsets with true-F32 matmuls on
        # gathered patches, and scatter-ADD them into the zeroed rows.
        def Y(ap):  # y-axis slice of [128, 25*18] -> [128, 25, 9]
            return ap[:].rearrange("p (t k) -> p t k", k=18)[:, :, 0:9]

        def X(ap):
            return ap[:].rearrange("p (t k) -> p t k", k=18)[:, :, 9:18]

        def V9(ap):  # [128, 25*9] -> [128, 25, 9]
            return ap[:].rearrange("p (t k) -> p t k", k=9)

        if USE_FIXUP:
            TH = 1e-3
            psF = psCctx.enter_context(tc.tile_pool(name="psF", bufs=1, space="PSUM"))
            d_offd = dpool.tile([4096, 64], F32, name="d_offd")
            d_ftab = dpool.tile([256, 64], F32, name="d_ftab")

            def Y(ap):  # y-axis slice of [128, 25*18] -> [128, 25, 9]
                return ap[:].rearrange("p (t k) -> p t k", k=18)[:, :, 0:9]

            def X(ap):
                return ap[:].rearrange("p (t k) -> p t k", k=18)[:, :, 9:18]

            XV = X  # x-offset view [128, 25, 9] of [128, 25*18]

            def V9(ap):  # [128, 25*9] -> [128, 25, 9]
                return ap[:].rearrange("p (t k) -> p t k", k=9)

            fr = sb.tile([C, NTILE * NTAP], F32, tag="fxfr")
            fri = sb.tile([C, NTILE * NTAP], I32, tag="fxfri")
            fl2 = sb.tile([C, NTILE * NTAP], F32, tag="fxfl2")
            fl = sb.tile([C, NTILE], F32)
            rank = sb.tile([C, NTILE], F32)
            svec = sb.tile([C, 1], F32, tag="fxs")
            pbase = sb.tile([C, 1], F32)
            # fr = frac(ox) via exact floor; near-integer iff fr < TH or fr > 1-TH
            nc.vector.tensor_copy(out=V9(fri), in_=XV(offT))
            nc.vector.tensor_copy(out=V9(fr), in_=V9(fri))
            nc.vector.tensor_tensor(V9(fl2), XV(offT), V9(fr), op=ALU.is_lt)
            nc.vector.tensor_tensor(V9(fr), V9(fr), V9(fl2), op=ALU.subtract)
            nc.vector.tensor_tensor(V9(fr), XV(offT), V9(fr), op=ALU.subtract)
            nc.vector.tensor_scalar(V9(fl2), V9(fr), TH, None, op0=ALU.is_lt)
            nc.vector.tensor_scalar(V9(fr), V9(fr), 1.0 - TH, None, op0=ALU.is_gt)
            nc.vector.tensor_tensor(V9(fl2), V9(fl2), V9(fr), op=ALU.max)
            nc.vector.tensor_reduce(
                out=fl[:].rearrange("p (t u) -> p t u", u=1),
                in_=V9(fl2), axis=mybir.AxisListType.X, op=ALU.max)
            # zero flagged pixels' x-offsets in place
            flb = fl2  # reuse
            nc.vector.tensor_scalar(flb[:, :NTILE], fl[:], -1.0, 1.0,
                                    op0=ALU.mult, op1=ALU.add)
            flbv = flb[:, :NTILE].rearrange("p (t u) -> p t u", u=1)
            flbb = bass.AP(tensor=flbv.tensor, offset=flbv.offset,
                           ap=[list(flbv.ap[0]), list(flbv.ap[1]), [0, NTAP]])
            nc.vector.tensor_tensor(XV(offT), XV(offT), flbb, op=ALU.mult)
            # offsets -> DRAM pixel rows (row = t*128+p, 64-f32 stride)
            od_w = d_offd[:].rearrange("(t p) c -> p t c", p=C)[:, :NTILE, 0:18]
            nc.sync.dma_start(od_w, offT[:])
            # ranks: pbase[p] = sum of flags on partitions < p; + exclusive scan
            nc.vector.tensor_reduce(out=svec[:], in_=fl[:],
                                    axis=mybir.AxisListType.X, op=ALU.add)
            psL = psF.tile([C, 1], F32, tag="psL")
            nc.tensor.matmul(psL[:], tri[:], svec[:], start=True, stop=True)
            nc.scalar.activation(pbase[:], psL[:], ACTF.Copy)
            nc.vector.tensor_tensor_scan(rank[:], fl[:], fl[:], initial=0.0,
                                         op0=ALU.add, op1=ALU.max)
            nc.vector.tensor_scalar(rank[:], rank[:], pbase[:, :1], None,
                                    op0=ALU.add)
            nc.vector.tensor_tensor(rank[:], rank[:], fl[:], op=ALU.subtract)
            nc.vector.tensor_scalar(rank[:], rank[:], 127.0, None, op0=ALU.min)
            # unflagged pixels -> dump slot 128: every live slot gets exactly one
            # add (concurrent scatter-adds to one address lose updates)
            nc.vector.tensor_scalar(rank[:], rank[:], -128.0, None, op0=ALU.add)
            nc.vector.tensor_tensor(rank[:], rank[:], fl[:], op=ALU.mult)
            nc.vector.tensor_scalar(rank[:], rank[:], 128.0, None, op0=ALU.add)
            # scatter fl*(pixid+1) into the 128-slot table at rank
            vtab = sb.tile([C, NTILE], F32, tag="fxv")
            nc.vector.tensor_tensor(vtab[:], fl[:], pixid1[:], op=ALU.mult)
            rank16 = sb.tile([C, NTILE], I16)
            nc.vector.tensor_copy(out=rank16[:], in_=rank[:])
            rwr = sb.tile([C, 200], I16)
            for k in range(8):
                dstr = rwr[0:16, :].rearrange("p (t k) -> p t k", k=8)[:, :, k:k + 1]
                nc.sync.dma_start(dstr, rank16[16 * k:16 * k + 16, :].rearrange(
                    "p (t u) -> p t u", u=1))
            for lo, hi in ((16, 32), (32, 64), (64, 128)):
                nc.sync.dma_start(rwr[lo:hi, :], rwr[0:hi - lo, :])
            zt = sb.tile([C, 1], F32, tag="fxz")
            nc.vector.memset(zt[:], 0.0)
            ft_head = d_ftab[:, 0:1]
            nc.sync.dma_start(d_ftab[0:C, 0:1], zt[:])
            nc.gpsimd.dma_scatter_add(
                out_ap=ft_head,
                in_ap=vtab[:].rearrange("p (a u) -> p a u", u=1),
                idxs_ap=rwr[:, :],
                num_idxs=NTILE * C, num_idxs_reg=NTILE * C,
                elem_size=1, elem_step=64)
            # read table (wrapped 16x8), derive patch-run and scatter indices
            tb = sb.tile([16, 8], F32)
            tb_src = d_ftab[0:C, :].rearrange("(c r) u -> r c u", r=16)[:, :, 0:1]
            nc.sync.dma_start(tb[:], tb_src)
            pixv = sb.tile([16, 8], F32, tag="fxp")
            jj = sb.tile([16, 8], F32, tag="fxj")
            ji = sb.tile([16, 8], I32, tag="fxji")
            sc16 = sb.tile([C, 8], I16)
            nc.vector.tensor_scalar(pixv[:], tb[:], 3200.0, -1.0,
                                    op0=ALU.min, op1=ALU.add)
            nc.vector.tensor_copy(out=sc16[0:16, :], in_=pixv[:])  # -1 pads end
            for lo, hi in ((16, 32), (32, 64), (64, 128)):
                nc.sync.dma_start(sc16[lo:hi, :], sc16[0:hi - lo, :])
            nc.vector.tensor_scalar(pixv[:], pixv[:], 0.0, None, op0=ALU.max)
            nc.vector.tensor_scalar(jj[:], pixv[:], 1.0 / 56, 1e-4,
                                    op0=ALU.mult, op1=ALU.add)
            nc.vector.tensor_copy(out=ji[:], in_=jj[:])
            nc.vector.tensor_copy(out=jj[:], in_=ji[:])  # == floor (args >= 0)
            # rbase = 58*i + j = 58*pix - 3247*j  (i = pix - 56*j)
            nc.vector.tensor_scalar(jj[:], jj[:], -3247.0, None, op0=ALU.mult)
            nc.vector.tensor_scalar(pixv[:], pixv[:], 58.0, None, op0=ALU.mult)
            nc.vector.tensor_tensor(pixv[:], pixv[:], jj[:], op=ALU.add)
            pidxf = sb.tile([16, 24], F32, tag="fxpi")
            for ky in range(3):
                nc.vector.tensor_scalar(pidxf[:, ky * 8:(ky + 1) * 8], pixv[:],
                                        58.0 * ky, None, op0=ALU.add)
            pidx = sb.tile([C, 24], I16)
            nc.vector.tensor_copy(out=pidx[0:16, :], in_=pidxf[:])
            for lo, hi in ((16, 32), (32, 64), (64, 128)):
                nc.sync.dma_start(pidx[lo:hi, :], pidx[0:hi - lo, :])
            # gather 3x3-row patches (3 one-row-triple runs per flagged pixel)
            gk = sb.tile([C, 3, 3 * C], F32)
            xpm_runs = bass.AP(tensor=d_xpm.tensor, offset=0,
                               ap=[[C, HP * HP - 2], [1, 3 * C]])
            for ky in range(3):
                nc.gpsimd.dma_gather(
                    out_ap=gk[:, ky:ky + 1, :], in_ap=xpm_runs,
                    idxs_ap=pidx[:, ky * 8:(ky + 1) * 8],
                    num_idxs=C, num_idxs_reg=C,
                    elem_size=3 * C, elem_step=C)
            # transpose patches to channel-major, precise F32 conv, add bias
            patchf = sb.tile([C, NTAP * C], F32)
            for tap in range(NTAP):
                ky, kx = tap // 3, tap % 3
                psK = psF.tile([C, C], F32, tag="psK")
                nc.tensor.transpose(psK[:], gk[:, ky, kx * C:(kx + 1) * C],
                                    ident[:])
                nc.scalar.activation(patchf[:, tap * C:(tap + 1) * C], psK[:],
                                     ACTF.Copy)
            psP = psF.tile([18, C], F32, tag="psP")
            for tap in range(NTAP):
                nc.tensor.matmul(psP[:], offw[:, tap * 18:(tap + 1) * 18],
                                 patchf[:, tap * C:(tap + 1) * C],
                                 start=(tap == 0), stop=(tap == NTAP - 1))
            prec = sb.tile([18, C], F32)
            nc.scalar.activation(prec[:], psP[:], ACTF.Identity,
                                 bias=offb[:, :1], scale=1.0)
            psQ = psF.tile([C, 18], F32, tag="psQ")
            nc.tensor.transpose(psQ[:], prec[:], ident[:18, :18])
            delta = sb.tile([C, 18], F32)
            nc.scalar.activation(delta[:], psQ[:], ACTF.Copy)
            # scatter x-offsets into the zeroed rows (idx < 0 at the end ignored)
            od_x = d_offd[:, 9:18]
            nc.gpsimd.dma_scatter_add(
                out_ap=od_x,
                in_ap=delta[:, 9:18].rearrange("p (a u) -> p a u", a=1),
                idxs_ap=sc16[:, :], num_idxs=C, num_idxs_reg=C,
                elem_size=9, elem_step=64)
            # corrected offsets back to SBUF
            offT2 = sb.tile([C, NTILE * 18], F32)
            nc.sync.dma_start(offT2[:], od_w)
            if DEBUG_FIX:
                nc.sync.dma_start(d["dbg_offTm"][:], offT[:])
                nc.sync.dma_start(d["dbg_offT2"][:], offT2[:])
                nc.sync.dma_start(d["dbg_fl"][:], fl[:])
                nc.sync.dma_start(d["dbg_rank"][:], rank[:])
                nc.sync.dma_start(d["dbg_tb"][:], tb[:])
                nc.sync.dma_start(d["dbg_delta"][:], delta[:])
                nc.sync.dma_start(d["dbg_prec"][:], prec[:])
                nc.sync.dma_start(d["dbg_gk"][:],
                                  gk[:].rearrange("p a b -> p (a b)"))

        else:
            offT2 = offT

        # ---- D: index + weight math ----
        # layout [128, 25*18]: col (t*18 + k), k in 0..8 = y taps, 9..17 = x taps
        w_lt = sb.tile([C, NTILE * NTAP], F32)
        w_rb = sb.tile([C, NTILE * NTAP], F32)
        w_lb = sb.tile([C, NTILE * NTAP], F32)
        w_rt = sb.tile([C, NTILE * NTAP], F32)
        s0f = sb.tile([C, NTILE * NTAP], F32)
        s1f = sb.tile([C, NTILE * NTAP], F32)
        idxf = sb.tile([C, NTILE * NTAP], F32)

        tmp = sb.tile([C, NTILE * 18], F32, tag="dtmp")      # p
        q = sb.tile([C, NTILE * 18], F32, tag="dtmp2")       # q = floor(p)
        qlt = sb.tile([C, NTILE * 18], F32, tag="dtmp3")
        qrb = sb.tile([C, NTILE * 18], F32, tag="dtmp4")
        pc = sb.tile([C, NTILE * 18], F32, tag="dtmp5")
        gA = sb.tile([C, NTILE * 18], F32, tag="dtmp6")      # 1 - f
        hh = sb.tile([C, NTILE * 18], F32, tag="dtmp7")      # 1 - (qrb - pc)
        t0 = sb.tile([C, NTILE * NTAP], F32, tag="dtmp8")
        t1 = sb.tile([C, NTILE * NTAP], F32, tag="dtmp9")
        tt = sb.tile([C, NTILE * NTAP], F32, tag="dtmp10")

        # per-half chain: y-half runs on the fast offsets (untouched by the
        # fixup) and overlaps the fixup's DMA latency; x-half waits for offT2
        ti = sb.tile([C, NTILE * 18], I32, tag="dti")

        def halfchain(V, src):
            nc.vector.tensor_tensor(V(tmp), V(base), V(src), op=ALU.add)
            nc.vector.tensor_copy(out=V(ti), in_=V(tmp))
            nc.vector.tensor_copy(out=V(q), in_=V(ti))
            nc.vector.tensor_tensor(V(qlt), V(tmp), V(q), op=ALU.is_lt)
            nc.vector.tensor_tensor(V(q), V(q), V(qlt), op=ALU.subtract)
            nc.vector.tensor_scalar(V(qlt), V(q), 0.0, 57.0,
                                    op0=ALU.max, op1=ALU.min)
            nc.vector.tensor_scalar(V(qrb), V(q), 1.0, 0.0,
                                    op0=ALU.add, op1=ALU.max)
            nc.vector.tensor_scalar(V(qrb), V(qrb), 57.0, None, op0=ALU.min)
            nc.vector.tensor_scalar(V(pc), V(tmp), 0.0, 57.0,
                                    op0=ALU.max, op1=ALU.min)
            nc.vector.tensor_tensor(V(gA), V(pc), V(qlt), op=ALU.subtract)
            nc.vector.tensor_scalar(V(gA), V(gA), -1.0, 1.0,
                                    op0=ALU.mult, op1=ALU.add)
            nc.vector.tensor_tensor(V(hh), V(qrb), V(pc), op=ALU.subtract)
            nc.vector.tensor_scalar(V(hh), V(hh), -1.0, 1.0,
                                    op0=ALU.mult, op1=ALU.add)

        halfchain(Y, offT)
        halfchain(X, offT2)
        # trunc factors (x axis): t0 = (gA_x >= 1), t1 = (hh_x >= 1)
        nc.vector.tensor_scalar(V9(t0), X(gA), 1.0, None, op0=ALU.is_ge)
        nc.vector.tensor_scalar(V9(t1), X(hh), 1.0, None, op0=ALU.is_ge)
        # weights
        nc.vector.tensor_tensor(V9(w_lt), Y(gA), V9(t0), op=ALU.mult)
        nc.vector.tensor_tensor(V9(w_rb), Y(hh), V9(t1), op=ALU.mult)
        nc.vector.tensor_tensor(V9(w_lb), Y(gA), X(hh), op=ALU.mult)
        nc.vector.tensor_tensor(V9(w_rt), Y(hh), X(gA), op=ALU.mult)
        # interior slot weights (anti-diagonal pair + exact-hit fold):
        #   s0 = w_lt + w_rt*(1-t0)   (slot0 = bl normally, lt at exact hit)
        #   s1 = w_lb + t0*(w_rt - w_lb)  (slot1 = tr normally, bl at hit)
        nc.vector.tensor_tensor(tt[:], t0[:], w_rt[:], op=ALU.mult)
        nc.vector.tensor_tensor(s0f[:], w_lt[:], w_rt[:], op=ALU.add)
        nc.vector.tensor_tensor(s0f[:], s0f[:], tt[:], op=ALU.subtract)
        nc.vector.tensor_tensor(s1f[:], w_rt[:], w_lb[:], op=ALU.subtract)
        nc.vector.tensor_tensor(s1f[:], s1f[:], t0[:], op=ALU.mult)
        nc.vector.tensor_tensor(s1f[:], s1f[:], w_lb[:], op=ALU.add)
        s0, s1 = s0f, s1f  # scalar operands must stay f32
        wltb, wrbb, wlbb, wrtb = w_lt, w_rb, w_lb, w_rt
        # gather base index: s = clip(q, -1, 57); e = sy*60 + sx + 61
        # idx rows (C-units) of xr2: edge tiles 2e; interior 2e + 1 - t0
        nc.vector.tensor_scalar(q[:], q[:], -1.0, 57.0, op0=ALU.max, op1=ALU.min)
        nc.vector.tensor_scalar(V9(idxf), Y(q), 120.0, 122.0,
                                op0=ALU.mult, op1=ALU.add)
        nc.vector.tensor_tensor(V9(idxf), V9(idxf), X(q), op=ALU.add)
        nc.vector.tensor_tensor(V9(idxf), V9(idxf), X(q), op=ALU.add)
        # interior tiles: idx += 1 - t0
        iv = idxf[:].rearrange("p (t k) -> p t k", k=9)[:, INT_T0:INT_T1, :]
        tv = t0[:].rearrange("p (t k) -> p t k", k=9)[:, INT_T0:INT_T1, :]
        nc.vector.tensor_scalar(iv, iv, 1.0, None, op0=ALU.add)
        nc.vector.tensor_tensor(iv, iv, tv, op=ALU.subtract)
        # int16 idx in per-tap layout [edge0 | edge23 | edge24 | interior 1..22]
        # (8 wrapped cols per tile) via wrap DMAs:
        #   interior: widx[r, n*200 + 24 + 8(t-1) + k] = idx16[16k + r, t*9 + n]
        idx16 = sb.tile([C, NTILE * NTAP], I16)
        nc.vector.tensor_copy(out=idx16[:], in_=idxf[:])
        # (n, t)-major splits: interior col n*22 + (t-1); edge col n*3 + ei
        idx16i = sb.tile([C, NTAP * 22], I16)
        idx16e = sb.tile([C, NTAP * 3], I16)
        i3 = idx16[:].rearrange("p (t n) -> p t n", t=NTILE)
        nc.vector.tensor_copy(
            out=idx16i[:].rearrange("p (n s) -> p s n", s=22),
            in_=i3[:, INT_T0:INT_T1, :])
        nc.vector.tensor_copy(
            out=idx16e[:].rearrange("p (n s) -> p s n", s=3)[:, 0:1, :],
            in_=i3[:, 0:1, :])
        nc.vector.tensor_copy(
            out=idx16e[:].rearrange("p (n s) -> p s n", s=3)[:, 1:3, :],
            in_=i3[:, 23:25, :])
        widx_i = sb.tile([C, NTAP * 176], I16)   # interior: n*176 + 8(t-1) + k
        widx_e = sb.tile([C, NTAP * 24], I16)    # edge: n*24 + 8*ei + k
        for k in range(8):
            dsti = widx_i[0:16, :].rearrange(
                "p (s k) -> p s k", k=8)[:, :, k:k + 1]
            dste = widx_e[0:16, :].rearrange(
                "p (s k) -> p s k", k=8)[:, :, k:k + 1]
            nc.sync.dma_start(dsti, idx16i[16 * k:16 * k + 16, :].rearrange(
                "p (s u) -> p s u", u=1))
            nc.sync.dma_start(dste, idx16e[16 * k:16 * k + 16, :].rearrange(
                "p (s u) -> p s u", u=1))
        for lo, hi in ((16, 32), (32, 64), (64, 128)):
            nc.sync.dma_start(widx_i[lo:hi, :], widx_i[0:hi - lo, :])
            nc.sync.dma_start(widx_e[lo:hi, :], widx_e[0:hi - lo, :])

        # ---- E: per-tap gather + blend + transpose + conv ----
        psCctx.close()  # release phase-C PSUM banks
        psT = ctx.enter_context(tc.tile_pool(name="psT", bufs=2, space="PSUM"))
        psO = ctx.enter_context(tc.tile_pool(name="psO", bufs=1, space="PSUM"))
        accs = [psO.tile([OUT, 512], F32, tag=f"acc{ch}", name=f"acc{ch}")
                for ch in range(6)]
        out6 = sb.tile([OUT, C], F32)  # SBUF accumulator for pixels 3072:3200

        # xr2 viewed as overlapping runs: row i = elements [i*C, i*C + len)
        xr_pair = bass.AP(tensor=d_xr2.tensor, offset=0,
                          ap=[[C, XR_ROWS - 1], [1, 2 * C]])
        xr_quad = bass.AP(tensor=d_xr2.tensor, offset=0,
                          ap=[[C, XR_ROWS - 3], [1, 4 * C]])
        for tap in range(NTAP):
            g2 = gpool.tile([C, NTILE, 2 * C], BF16, tag="g2", name="g2")
            g4 = gpool.tile([C, 4, 4 * C], BF16, tag="g4", name="g4")
            # edge tiles 0,23,24: 4-corner quad, 1 desc/(pix,tap)
            nc.gpsimd.dma_gather(
                out_ap=g4[:, 0:3, :], in_ap=xr_quad,
                idxs_ap=widx_e[:, tap * 24: tap * 24 + 24],
                num_idxs=384, num_idxs_reg=384,
                elem_size=4 * C, elem_step=C)
            # interior tiles 1..22: anti-diagonal pair, 1 desc/(pix,tap);
            # chunked to fit the default 1024-entry SWDGE ring
            for j0, nj in ((0, 1024), (1024, 1024), (2048, 768)):
                c0 = tap * 176 + j0 // 16
                nc.gpsimd.dma_gather(
                    out_ap=g2[:, INT_T0 + j0 // 128: INT_T0 + (j0 + nj) // 128, :],
                    in_ap=xr_pair,
                    idxs_ap=widx_i[:, c0: c0 + nj // 16],
                    num_idxs=nj, num_idxs_reg=nj,
                    elem_size=2 * C, elem_step=C)

            rhs = rpool.tile([C, NPAD], BF16, tag="rhs")
            for tq in range(7):  # quads of pixel-tiles
                ntq = 4 if tq < 6 else 1
                pst = psT.tile([C, 512], F32, tag="pstr")
                pstb = pst[:].bitcast(BF16)
                for k in range(ntq):
                    t = tq * 4 + k
                    wcol = slice(t * NTAP + tap, t * NTAP + tap + 1)
                    xo = xpool.tile([C, C], BF16, tag="xo")
                    ve = nc.vector
                    if t in EDGE_T:
                        ei = 0 if t == 0 else t - 22
                        ve.tensor_scalar(
                            xo[:], g4[:, ei, 0:C], wltb[:, wcol], None,
                            op0=ALU.mult)
                        ve.scalar_tensor_tensor(
                            xo[:], g4[:, ei, C:2 * C], wrtb[:, wcol], xo[:],
                            op0=ALU.mult, op1=ALU.add)
                        ve.scalar_tensor_tensor(
                            xo[:], g4[:, ei, 2 * C:3 * C], wlbb[:, wcol], xo[:],
                            op0=ALU.mult, op1=ALU.add)
                        ve.scalar_tensor_tensor(
                            xo[:], g4[:, ei, 3 * C:4 * C], wrbb[:, wcol], xo[:],
                            op0=ALU.mult, op1=ALU.add)
                    else:
                        ve.tensor_scalar(
                            xo[:], g2[:, t, 0:C], s0[:, wcol], None,
                            op0=ALU.mult)
                        ve.scalar_tensor_tensor(
                            xo[:], g2[:, t, C:2 * C], s1[:, wcol], xo[:],
                            op0=ALU.mult, op1=ALU.add)
                    nc.tensor.transpose(pstb[:, k * C:(k + 1) * C], xo[:],
                                        identb[:])
                nc.scalar.activation(rhs[:, tq * 512: tq * 512 + ntq * C],
                                     pstb[:, :ntq * C], ACTF.Copy)

            for ch in range(6):
                nc.tensor.matmul(
                    accs[ch][:],
                    convw[:, tap * OUT:(tap + 1) * OUT],
                    rhs[:, 512 * ch: 512 * ch + 512],
                    start=(tap == 0), stop=(tap == NTAP - 1))
            ps6 = psT.tile([C, 512], F32, tag="pstr")
            nc.tensor.matmul(ps6[:, :C],
                             convw[:, tap * OUT:(tap + 1) * OUT],
                             rhs[:, 3072:3200],
                             start=True, stop=True)
            if tap == 0:
                nc.vector.tensor_copy(out=out6[:], in_=ps6[:, :C])
            else:
                nc.vector.tensor_tensor(out6[:], out6[:], ps6[:, :C],
                                        op=ALU.add)

        # ---- F: output ----
        for ch in range(6):
            ob = opool.tile([OUT, 512], F32, tag="ob")
            nc.scalar.activation(ob[:], accs[ch][:], ACTF.Copy)
            nc.sync.dma_start(d_out[:, 512 * ch:512 * ch + 512], ob[:])
        nc.sync.dma_start(d_out[:, 3072:3200], out6[:])


# ---------------- host-side input prep ----------------

def prep_core_inputs(xb, offset_w, offset_b, conv_w):
    """Build the per-core in_map from one batch image [C, H, W] + weights."""
    f32 = np.float32
    xb = np.asarray(xb, f32)
    xp = np.pad(xb, ((0, 0), (1, 1), (1, 1)))                   # [C, 58, 58]
    xcp = np.ascontiguousarray(xp.reshape(C, HP * HP))
    xpm = np.ascontiguousarray(xp.transpose(1, 2, 0).reshape(HP * HP, C))
    xr60 = np.pad(xp, ((0, 0), (1, 1), (1, 1)), mode="edge")    # [C, 60, 60]
    xr60 = xr60.transpose(1, 2, 0)                              # [60, 60, C]
    # interleaved row pairs: xr2[2*(y*60+x)] = xr60[y,x]; [.. +1] = xr60[y+1,x]
    xr2 = np.stack([xr60[:-1], xr60[1:]], axis=2)               # [59, 60, 2, C]
    xr2 = np.ascontiguousarray(xr2.reshape(XR_ROWS, C)).astype(ml_dtypes.bfloat16)

    offw = np.empty((C, NTAP * 18), f32)
    convw = np.empty((C, NTAP * OUT), f32)
    for tap in range(NTAP):
        ky, kx = tap // 3, tap % 3
        offw[:, tap * 18:(tap + 1) * 18] = np.asarray(offset_w, f32)[:, :, ky, kx].T
        convw[:, tap * OUT:(tap + 1) * OUT] = np.asarray(conv_w, f32)[:, :, ky, kx].T
    offb = np.asarray(offset_b, f32).reshape(18, 1)

    # base grid [128, 25*18]: partition p, col t*18+k -> pixel t*128+p (col-major)
    r = np.arange(-1, 2, dtype=f32)
    py_n, px_n = np.meshgrid(r, r, indexing="ij")
    pny, pnx = py_n.ravel(), px_n.ravel()
    gy = np.arange(1, 57, dtype=f32)
    p0y, p0x = np.meshgrid(gy, gy, indexing="ij")
    p0yc, p0xc = p0y.T.ravel(), p0x.T.ravel()      # col-major pixels
    base = np.empty((NPAD, 18), f32)
    base[:NPIX, :NTAP] = p0yc[:, None] + pny[None, :]
    base[:NPIX, NTAP:] = p0xc[:, None] + pnx[None, :]
    base[NPIX:, :NTAP] = 28.0 + pny[None, :]
    base[NPIX:, NTAP:] = 28.0 + pnx[None, :]
    base = np.ascontiguousarray(
        base.reshape(NTILE, C, 18).transpose(1, 0, 2).reshape(C, NTILE * 18))

    tri = np.triu(np.ones((C, C), f32), 1)  # tri[p, m] = 1 iff p < m
    pixid1 = np.ascontiguousarray(
        (np.arange(NTILE)[None, :] * C + np.arange(C)[:, None] + 1)
        .astype(f32))
    return {"xcp": xcp, "xpm": xpm, "xr2": xr2, "offw": offw, "offb": offb,
            "convw": convw.astype(ml_dtypes.bfloat16), "base": base,
            "tri": tri, "pixid1": pixid1}


def postprocess(out_np):
    """[OUT, 3200] col-major -> [OUT, 56, 56]."""
    o = out_np[:, :NPIX].reshape(OUT, W, H).transpose(0, 2, 1)
    return np.ascontiguousarray(o)


# ---------------- entry point ----------------

N_CORES = 8
_cache = {}


def _build():
    if "nc" in _cache:
        return _cache["nc"]
    nc = bacc.Bacc("TRN2", target_bir_lowering=False, debug=False,
                   enable_asserts=True, num_devices=N_CORES)
    build_kernel(nc)
    nc.compile()
    nc.m = get_hw_module(nc.m)
    _cache["nc"] = nc
    return nc


def kernel(x, offset_w, offset_b, conv_w):
    x = np.asarray(x, np.float32)
    assert x.shape == (N_CORES, C, H, W), x.shape
    nc = _build()
    in_maps = [prep_core_inputs(x[b], offset_w, offset_b, conv_w)
               for b in range(N_CORES)]
    res = run_bass_kernel_spmd(nc, in_maps, core_ids=list(range(N_CORES)))
    outs = [postprocess(res.results[b]["out"]) for b in range(N_CORES)]
    return np.stack(outs).astype(np.float32)



# revision 21
# speedup vs baseline: 1.2772x; 1.2772x over previous
"""Deformable-conv kernel for Trainium2: 8-core data-parallel over batch.

kernel(x, offset_w, offset_b, conv_w) -> [8, 128, 56, 56] float32.
Each NeuronCore processes one batch image:
  offset conv in true-F32 PE matmuls (the reference sampler is discontinuous
  at integer x-coords, so offsets need ~1e-7 accuracy to reproduce its
  floor/trunc decisions) -> pixel-partitioned offsets (PE transpose)
  -> index/bilinear-weight math (DVE) -> bf16 indirect-DMA gathers from a
  row-pair-interleaved padded map (interior pixels: one 512B descriptor per
  (pixel, tap) fetching the anti-diagonal [bot-left, top-right] corner pair,
  index shifted by the exact-integer-hit mask; edge tiles: one 1KB 4-corner
  descriptor) -> 2-term (interior) / 4-term (edge) blend (DVE) -> bf16 PE
  transpose -> 3x3/stride-3 conv as 9 accumulating bf16 matmuls (PSUM).
"""
import sys
for _p in ("/opt/trn_rl_repo", "/root/.axon_site/_ro/trn_rl_repo"):
    if _p not in sys.path:
        sys.path.append(_p)

from contextlib import ExitStack

import numpy as np
import ml_dtypes

import concourse.bass as bass
import concourse.bacc as bacc
import concourse.mybir as mybir
import concourse.tile as tile
from concourse.masks import make_identity
from concourse.bass_utils import run_bass_kernel_spmd
from concourse.bass_interp import get_hw_module

F32 = mybir.dt.float32
BF16 = mybir.dt.bfloat16
I32 = mybir.dt.int32
I16 = mybir.dt.int16
ALU = mybir.AluOpType
ACTF = mybir.ActivationFunctionType

DEBUG_FIX = False
USE_FIXUP = False
USE_F32R = False
FIXLEVEL = 5  # bisect knob: 1=flag+zero+dram rt, 2=+rank/table, 3=+patch gather,
              # 4=+precise conv, 5=full (delta scatter-add)
H = W = 56
HP = 58
NPIX = H * W          # 3136
NPAD = 3200           # padded pixel count (25 tiles of 128)
NTILE = 25
NTAP = 9
C = 128
OUT = 128
XR_ROWS = 3540 * 2    # interleaved row-pair map: entry e -> rows 2e, 2e+1
# edge tiles: pixel cols j<=2 or j>=52 live here (clip/trunc can fire in x)
EDGE_T = (0, 23, 24)
INT_T0, INT_T1 = 1, 23  # interior tiles [1, 23)


def build_kernel(nc):
    d = {
        "xcp": nc.dram_tensor("xcp", [C, HP * HP], F32, kind="ExternalInput").ap(),
        "xpm": nc.dram_tensor("xpm", [HP * HP, C], F32, kind="ExternalInput").ap(),
        "xr2": nc.dram_tensor("xr2", [XR_ROWS, C], BF16, kind="ExternalInput").ap(),
        "offw": nc.dram_tensor("offw", [C, NTAP * 18], F32, kind="ExternalInput").ap(),
        "offb": nc.dram_tensor("offb", [18, 1], F32, kind="ExternalInput").ap(),
        "convw": nc.dram_tensor("convw", [C, NTAP * OUT], BF16, kind="ExternalInput").ap(),
        "base": nc.dram_tensor("base", [C, NTILE * 18], F32, kind="ExternalInput").ap(),
        "tri": nc.dram_tensor("tri", [C, C], F32, kind="ExternalInput").ap(),
        "pixid1": nc.dram_tensor("pixid1", [C, NTILE], F32, kind="ExternalInput").ap(),
        "out": nc.dram_tensor("out", [OUT, NPAD], F32, kind="ExternalOutput").ap(),
    }
    if DEBUG_FIX:
        for nm, shp, dt in (("dbg_offTm", [C, NTILE * 18], F32),
                            ("dbg_offT2", [C, NTILE * 18], F32),
                            ("dbg_fl", [C, NTILE], F32),
                            ("dbg_rank", [C, NTILE], F32),
                            ("dbg_tb", [16, 16], F32),
                            ("dbg_delta", [C, 2 * 18], F32),
                            ("dbg_prec", [18, 256], F32),
                            ("dbg_gk", [C, 3 * 2 * 3 * C], F32)):
            d[nm] = nc.dram_tensor(nm, shp, dt, kind="ExternalOutput").ap()
    with tile.TileContext(nc) as tc:
        emit(tc, d)
    return nc


def emit(tc, d):
    d_xcp, d_xpm, d_xr2 = d["xcp"], d["xpm"], d["xr2"]
    d_offw, d_offb, d_convw = d["offw"], d["offb"], d["convw"]
    d_base, d_tri, d_pixid1, d_out = d["base"], d["tri"], d["pixid1"], d["out"]
    nc = tc.nc
    F32R = mybir.dt.float32r
    ctx = ExitStack()
    with ctx:
        consts = ctx.enter_context(tc.tile_pool(name="consts", bufs=1))
        sb = ctx.enter_context(tc.tile_pool(name="sb", bufs=1))
        gpool = ctx.enter_context(tc.tile_pool(name="gpool", bufs=3))
        xpool = ctx.enter_context(tc.tile_pool(name="xpool", bufs=8))
        rpool = ctx.enter_context(tc.tile_pool(name="rpool", bufs=2))
        opool = ctx.enter_context(tc.tile_pool(name="opool", bufs=2))
        dpool = ctx.enter_context(tc.tile_pool(name="dpool", bufs=1, space="DRAM"))
        psWctx = ExitStack()
        psW = psWctx.enter_context(tc.tile_pool(name="psW", bufs=1,
                                                space="PSUM"))
        psBctx = ExitStack()
        psB = psBctx.enter_context(tc.tile_pool(name="psB", bufs=1, space="PSUM"))

        # ---- A: loads ----
        xcpr = None
        if USE_FIXUP or USE_F32R:
            xcpr = consts.tile([C, HP * HP], F32R)
            nc.gpsimd.dma_start(xcpr[:], d_xcp[:])
        xcpf = consts.tile([C, HP * HP], F32)
        nc.sync.dma_start(xcpf[:], d_xcp[:])
        offw = consts.tile([C, NTAP * 18], F32)
        nc.sync.dma_start(offw[:], d_offw[:])
        offwr = None
        if USE_FIXUP or USE_F32R:
            offwr = consts.tile([C, NTAP * 18], F32R)
            nc.gpsimd.dma_start(offwr[:], d_offw[:])
        convw = consts.tile([C, NTAP * OUT], BF16)
        nc.sync.dma_start(convw[:], d_convw[:])
        offb = consts.tile([18, 1], F32)
        nc.sync.dma_start(offb[:], d_offb[:])
        base = consts.tile([C, NTILE * 18], F32)
        nc.sync.dma_start(base[:], d_base[:])
        if USE_FIXUP:
            tri = consts.tile([C, C], F32)
            nc.sync.dma_start(tri[:], d_tri[:])
            pixid1 = consts.tile([C, NTILE], F32)
            nc.sync.dma_start(pixid1[:], d_pixid1[:])
        ident = consts.tile([C, C], F32)
        make_identity(nc, ident[:])
        identb = consts.tile([C, C], BF16)
        nc.vector.tensor_copy(out=identb[:], in_=ident[:])

        # PE p-state warmup: ~70 cheap matmuls fill the PE queue for >3us of
        # busy time (hidden under the input DMA loads), so phase B's matmuls
        # dispatch with the tensor clock fully ramped (cost-model p-state).
        wz = consts.tile([C, 64], BF16)
        nc.vector.memset(wz[:], 0.0)
        psw = psW.tile([1, 64], F32, tag="psw", name="psw")
        for _ in range(70):
            nc.tensor.matmul(psw[:], identb[:, 0:1], wz[:],
                             start=True, stop=True)

        # ---- B: offset conv, fast F32R pass (fixed up below for pixels whose
        # x-offset lands near an integer, where the reference's trunc/floor
        # decisions are discontinuous) ----
        # col-major output pixels: chunk c covers j in [8c, 8c+8), all i.
        off_sb = sb.tile([18, NPAD], F32)
        xcp3 = (xcpr if (USE_FIXUP or USE_F32R) else xcpf)[:].rearrange(
            "p (y x) -> p y x", y=HP)
        pss = [psB.tile([18, 448], F32, tag=f"psB{ch}", name=f"psB{ch}")
               for ch in range(7)]
        for tap in range(NTAP):
            ky, kx = tap // 3, tap % 3
            for ch in range(7):
                rhs = xcp3[:, ky:ky + 56, kx + 8 * ch: kx + 8 * ch + 8] \
                    .transpose([0, 2, 1])
                lhsw = offwr if (USE_FIXUP or USE_F32R) else offw
                nc.tensor.matmul(
                    pss[ch][:], lhsw[:, tap * 18:(tap + 1) * 18], rhs,
                    start=(tap == 0), stop=(tap == NTAP - 1))
        for ch in range(7):
            nc.scalar.activation(off_sb[:, 448 * ch:448 * (ch + 1)], pss[ch][:],
                                 ACTF.Identity, bias=offb[:, :1], scale=1.0)
        # pad pixels: 0.5 keeps them far from the near-integer flag band
        nc.vector.memset(off_sb[:, NPIX:], 0.5)

        # ---- C: transpose offsets to pixel-partitioned ----
        psBctx.close()
        psCctx = ExitStack()
        psC = psCctx.enter_context(tc.tile_pool(name="psC", bufs=2, space="PSUM"))
        offT = sb.tile([C, NTILE * 18], F32)
        for t in range(NTILE):
            pst = psC.tile([C, 18], F32, tag="psC")
            nc.tensor.transpose(pst[:], off_sb[:, t * C:(t + 1) * C],
                                ident[:18, :18])
            nc.scalar.activation(offT[:, t * 18:(t + 1) * 18], pst[:],
                                 ACTF.Copy)

        # ---- B2: precise fixup of near-integer x-offsets ----
        # Flag pixels with any x-offset within TH of an integer, zero their
        # x-offsets, round-trip offsets through DRAM (64-f32-padded pixel
        # rows), recompute flagged pixels' offsets with true-F32 matmuls on
        # gathered patches, and scatter-ADD them into the zeroed rows.
        def Y(ap):  # y-axis slice of [128, 25*18] -> [128, 25, 9]
            return ap[:].rearrange("p (t k) -> p t k", k=18)[:, :, 0:9]

        def X(ap):
            return ap[:].rearrange("p (t k) -> p t k", k=18)[:, :, 9:18]

        def V9(ap):  # [128, 25*9] -> [128, 25, 9]
            return ap[:].rearrange("p (t k) -> p t k", k=9)

        if USE_FIXUP:
            # Both x- AND y-offsets near an integer make the reference's
            # corner/trunc decisions discontinuous (only the anti-diagonal
            # corner pair survives generically, and which rows it sits on
            # flips at every y-integer crossing), so flag BOTH halves and
            # recompute flagged pixels' full 18-offset vector exactly.
            TH = 1e-3
            NSLOT = 256  # flagged-pixel capacity (E[flags] ~ 134/image)
            psF = psCctx.enter_context(tc.tile_pool(name="psF", bufs=1, space="PSUM"))
            d_offd = dpool.tile([4096, 64], F32, name="d_offd")
            d_ftab = dpool.tile([512, 64], F32, name="d_ftab")

            def Y(ap):  # y-axis slice of [128, 25*18] -> [128, 25, 9]
                return ap[:].rearrange("p (t k) -> p t k", k=18)[:, :, 0:9]

            def X(ap):
                return ap[:].rearrange("p (t k) -> p t k", k=18)[:, :, 9:18]

            def V18(ap):  # [128, 25*18] -> [128, 25, 18]
                return ap[:].rearrange("p (t k) -> p t k", k=18)

            fr = sb.tile([C, NTILE * 18], F32, tag="fxfr")
            fri = sb.tile([C, NTILE * 18], I32, tag="fxfri")
            fl2 = sb.tile([C, NTILE * 18], F32, tag="fxfl2")
            fl = sb.tile([C, NTILE], F32)
            rank = sb.tile([C, NTILE], F32)
            svec = sb.tile([C, 1], F32, tag="fxs")
            pbase = sb.tile([C, 1], F32)
            # fr = frac(off) via exact floor; near-integer iff fr<TH or fr>1-TH
            nc.vector.tensor_copy(out=V18(fri), in_=V18(offT))
            nc.vector.tensor_copy(out=V18(fr), in_=V18(fri))
            nc.vector.tensor_tensor(V18(fl2), V18(offT), V18(fr), op=ALU.is_lt)
            nc.vector.tensor_tensor(V18(fr), V18(fr), V18(fl2), op=ALU.subtract)
            nc.vector.tensor_tensor(V18(fr), V18(offT), V18(fr), op=ALU.subtract)
            nc.vector.tensor_scalar(V18(fl2), V18(fr), TH, None, op0=ALU.is_lt)
            nc.vector.tensor_scalar(V18(fr), V18(fr), 1.0 - TH, None, op0=ALU.is_gt)
            nc.vector.tensor_tensor(V18(fl2), V18(fl2), V18(fr), op=ALU.max)
            nc.vector.tensor_reduce(
                out=fl[:].rearrange("p (t u) -> p t u", u=1),
                in_=V18(fl2), axis=mybir.AxisListType.X, op=ALU.max)
            # zero flagged pixels' offsets (all 18) in place
            flb = fl2  # reuse
            nc.vector.tensor_scalar(flb[:, :NTILE], fl[:], -1.0, 1.0,
                                    op0=ALU.mult, op1=ALU.add)
            flbv = flb[:, :NTILE].rearrange("p (t u) -> p t u", u=1)
            flbb = bass.AP(tensor=flbv.tensor, offset=flbv.offset,
                           ap=[list(flbv.ap[0]), list(flbv.ap[1]), [0, 18]])
            nc.vector.tensor_tensor(V18(offT), V18(offT), flbb, op=ALU.mult)
            # offsets -> DRAM pixel rows (row = t*128+p, 64-f32 stride)
            od_w = d_offd[:].rearrange("(t p) c -> p t c", p=C)[:, :NTILE, 0:18]
            nc.sync.dma_start(od_w, offT[:])
            # ranks: pbase[p] = sum of flags on partitions < p; + exclusive scan
            if FIXLEVEL >= 2:
                nc.vector.tensor_reduce(out=svec[:], in_=fl[:],
                                        axis=mybir.AxisListType.X, op=ALU.add)
                psL = psF.tile([C, 1], F32, tag="psL")
                nc.tensor.matmul(psL[:], tri[:], svec[:], start=True, stop=True)
                nc.scalar.activation(pbase[:], psL[:], ACTF.Copy)
                nc.vector.tensor_tensor_scan(rank[:], fl[:], fl[:], initial=0.0,
                                             op0=ALU.add, op1=ALU.max)
                nc.vector.tensor_scalar(rank[:], rank[:], pbase[:, :1], None,
                                        op0=ALU.add)
                nc.vector.tensor_tensor(rank[:], rank[:], fl[:], op=ALU.subtract)
                nc.vector.tensor_scalar(rank[:], rank[:], NSLOT - 1.0, None,
                                        op0=ALU.min)
                # unflagged pixels -> dump slot NSLOT: every live slot gets
                # exactly one add (concurrent adds to one address lose updates)
                nc.vector.tensor_scalar(rank[:], rank[:], -float(NSLOT), None,
                                        op0=ALU.add)
                nc.vector.tensor_tensor(rank[:], rank[:], fl[:], op=ALU.mult)
                nc.vector.tensor_scalar(rank[:], rank[:], float(NSLOT), None,
                                        op0=ALU.add)
                # scatter fl*(pixid+1) into the NSLOT-slot table at rank
                vtab = sb.tile([C, NTILE], F32, tag="fxv")
                nc.vector.tensor_tensor(vtab[:], fl[:], pixid1[:], op=ALU.mult)
                rank16 = sb.tile([C, NTILE], I16)
                nc.vector.tensor_copy(out=rank16[:], in_=rank[:])
                rwr = sb.tile([C, 200], I16)
                for k in range(8):
                    dstr = rwr[0:16, :].rearrange(
                        "p (t k) -> p t k", k=8)[:, :, k:k + 1]
                    nc.sync.dma_start(dstr, rank16[16 * k:16 * k + 16, :].rearrange(
                        "p (t u) -> p t u", u=1))
                for lo, hi in ((16, 32), (32, 64), (64, 128)):
                    nc.sync.dma_start(rwr[lo:hi, :], rwr[0:hi - lo, :])
                zt = sb.tile([C, 1], F32, tag="fxz")
                nc.vector.memset(zt[:], 0.0)
                ft_head = d_ftab[:, 0:1]
                nc.sync.dma_start(d_ftab[0:C, 0:1], zt[:])
                nc.sync.dma_start(d_ftab[C:2 * C, 0:1], zt[:])
                nc.gpsimd.dma_scatter_add(
                    out_ap=ft_head,
                    in_ap=vtab[:].rearrange("p (a u) -> p a u", u=1),
                    idxs_ap=rwr[:, :],
                    num_idxs=NTILE * C, num_idxs_reg=NTILE * C,
                    elem_size=1, elem_step=64)
            if FIXLEVEL >= 3:
                # read table (wrapped 16x16), derive patch-run + scatter indices
                tb = sb.tile([16, 16], F32)
                tb_src = d_ftab[0:NSLOT, :].rearrange(
                    "(c r) u -> r c u", r=16)[:, :, 0:1]
                nc.sync.dma_start(tb[:], tb_src)
                pixv = sb.tile([16, 16], F32, tag="fxp")
                jj = sb.tile([16, 16], F32, tag="fxj")
                ji = sb.tile([16, 16], I32, tag="fxji")
                sc16 = sb.tile([C, 16], I16)
                nc.vector.tensor_scalar(pixv[:], tb[:], 3200.0, -1.0,
                                        op0=ALU.min, op1=ALU.add)
                nc.vector.tensor_copy(out=sc16[0:16, :], in_=pixv[:])  # -1 pads
                for lo, hi in ((16, 32), (32, 64), (64, 128)):
                    nc.sync.dma_start(sc16[lo:hi, :], sc16[0:hi - lo, :])
                nc.vector.tensor_scalar(pixv[:], pixv[:], 0.0, None, op0=ALU.max)
                # i32 copy rounds-to-nearest; bias by -0.5+eps so round == floor
                nc.vector.tensor_scalar(jj[:], pixv[:], 1.0 / 56, 1e-4 - 0.5,
                                        op0=ALU.mult, op1=ALU.add)
                nc.vector.tensor_copy(out=ji[:], in_=jj[:])
                nc.vector.tensor_copy(out=jj[:], in_=ji[:])
                # rbase = 58*i + j = 58*pix - 3247*j  (i = pix - 56*j)
                nc.vector.tensor_scalar(jj[:], jj[:], -3247.0, None, op0=ALU.mult)
                nc.vector.tensor_scalar(pixv[:], pixv[:], 58.0, None, op0=ALU.mult)
                nc.vector.tensor_tensor(pixv[:], pixv[:], jj[:], op=ALU.add)
                pidxf = sb.tile([16, 48], F32, tag="fxpi")
                for ky in range(3):
                    nc.vector.tensor_scalar(pidxf[:, ky * 16:(ky + 1) * 16],
                                            pixv[:], 58.0 * ky, None, op0=ALU.add)
                pidx = sb.tile([C, 48], I16)
                nc.vector.tensor_copy(out=pidx[0:16, :], in_=pidxf[:])
                for lo, hi in ((16, 32), (32, 64), (64, 128)):
                    nc.sync.dma_start(pidx[lo:hi, :], pidx[0:hi - lo, :])
                # gather 3x3-row patches (3 one-row-triple runs per flagged pixel)
                gk = sb.tile([C, 3, 2, 3 * C], F32)
                xpm_runs = bass.AP(tensor=d_xpm.tensor, offset=0,
                                   ap=[[C, HP * HP - 2], [1, 3 * C]])
                for ky in range(3):
                    nc.gpsimd.dma_gather(
                        out_ap=gk[:, ky, :, :], in_ap=xpm_runs,
                        idxs_ap=pidx[:, ky * 16:(ky + 1) * 16],
                        num_idxs=NSLOT, num_idxs_reg=NSLOT,
                        elem_size=3 * C, elem_step=C)
            if FIXLEVEL >= 4:
                # transpose patches to channel-major, precise F32 conv, add bias
                patchf = sb.tile([C, NTAP * NSLOT], F32)
                for tap in range(NTAP):
                    ky, kx = tap // 3, tap % 3
                    for ch in range(2):
                        psK = psF.tile([C, C], F32, tag="psK")
                        nc.tensor.transpose(
                            psK[:], gk[:, ky, ch, kx * C:(kx + 1) * C], ident[:])
                        nc.scalar.activation(
                            patchf[:, tap * NSLOT + ch * C:
                                   tap * NSLOT + (ch + 1) * C],
                            psK[:], ACTF.Copy)
                psP = psF.tile([18, NSLOT], F32, tag="psP")
                for tap in range(NTAP):
                    nc.tensor.matmul(psP[:], offw[:, tap * 18:(tap + 1) * 18],
                                     patchf[:, tap * NSLOT:(tap + 1) * NSLOT],
                                     start=(tap == 0), stop=(tap == NTAP - 1))
                prec = sb.tile([18, NSLOT], F32)
                nc.scalar.activation(prec[:], psP[:], ACTF.Identity,
                                     bias=offb[:, :1], scale=1.0)
                delta = sb.tile([C, 2, 18], F32)
                for ch in range(2):
                    psQ = psF.tile([C, 18], F32, tag="psQ")
                    nc.tensor.transpose(psQ[:], prec[:, ch * C:(ch + 1) * C],
                                        ident[:18, :18])
                    nc.scalar.activation(delta[:, ch, :], psQ[:], ACTF.Copy)
            if FIXLEVEL >= 5:
                # scatter full 18-offset rows into the zeroed DRAM rows
                # (idx < 0 at the end ignored)
                od_x = d_offd[:, 0:18]
                nc.gpsimd.dma_scatter_add(
                    out_ap=od_x,
                    in_ap=delta[:, :, :],
                    idxs_ap=sc16[:, :], num_idxs=NSLOT, num_idxs_reg=NSLOT,
                    elem_size=18, elem_step=64)
            # corrected offsets back to SBUF
            offT2 = sb.tile([C, NTILE * 18], F32)
            nc.sync.dma_start(offT2[:], od_w)
            if DEBUG_FIX:
                nc.sync.dma_start(d["dbg_offTm"][:], offT[:])
                nc.sync.dma_start(d["dbg_offT2"][:], offT2[:])
                nc.sync.dma_start(d["dbg_fl"][:], fl[:])
                nc.sync.dma_start(d["dbg_rank"][:], rank[:])
                nc.sync.dma_start(d["dbg_tb"][:], tb[:])
                nc.sync.dma_start(d["dbg_delta"][:],
                                  delta[:].rearrange("p a b -> p (a b)"))
                nc.sync.dma_start(d["dbg_prec"][:], prec[:])
                nc.sync.dma_start(d["dbg_gk"][:],
                                  gk[:].rearrange("p a b c -> p (a b c)"))

        else:
            offT2 = offT

        # ---- D: index + weight math ----
        # layout [128, 25*18]: col (t*18 + k), k in 0..8 = y taps, 9..17 = x taps
        w_lt = sb.tile([C, NTILE * NTAP], F32)
        w_rb = sb.tile([C, NTILE * NTAP], F32)
        w_lb = sb.tile([C, NTILE * NTAP], F32)
        w_rt = sb.tile([C, NTILE * NTAP], F32)
        s0f = sb.tile([C, NTILE * NTAP], F32)
        s1f = sb.tile([C, NTILE * NTAP], F32)
        idxf = sb.tile([C, NTILE * NTAP], F32)

        tmp = sb.tile([C, NTILE * 18], F32, tag="dtmp")      # p
        q = sb.tile([C, NTILE * 18], F32, tag="dtmp2")       # q = floor(p)
        qlt = sb.tile([C, NTILE * 18], F32, tag="dtmp3")
        qrb = sb.tile([C, NTILE * 18], F32, tag="dtmp4")
        pc = sb.tile([C, NTILE * 18], F32, tag="dtmp5")
        gA = sb.tile([C, NTILE * 18], F32, tag="dtmp6")      # 1 - f
        hh = sb.tile([C, NTILE * 18], F32, tag="dtmp7")      # 1 - (qrb - pc)
        t0 = sb.tile([C, NTILE * NTAP], F32, tag="dtmp8")
        t1 = sb.tile([C, NTILE * NTAP], F32, tag="dtmp9")
        tt = sb.tile([C, NTILE * NTAP], F32, tag="dtmp10")

        # per-half chain: y-half runs on the fast offsets (untouched by the
        # fixup) and overlaps the fixup's DMA latency; x-half waits for offT2
        ti = sb.tile([C, NTILE * 18], I32, tag="dti")

        def halfchain(V, src):
            nc.vector.tensor_tensor(V(tmp), V(base), V(src), op=ALU.add)
            nc.vector.tensor_copy(out=V(ti), in_=V(tmp))
            nc.vector.tensor_copy(out=V(q), in_=V(ti))
            nc.vector.tensor_tensor(V(qlt), V(tmp), V(q), op=ALU.is_lt)
            nc.vector.tensor_tensor(V(q), V(q), V(qlt), op=ALU.subtract)
            nc.vector.tensor_scalar(V(qlt), V(q), 0.0, 57.0,
                                    op0=ALU.max, op1=ALU.min)
            nc.vector.tensor_scalar(V(qrb), V(q), 1.0, 0.0,
                                    op0=ALU.add, op1=ALU.max)
            nc.vector.tensor_scalar(V(qrb), V(qrb), 57.0, None, op0=ALU.min)
            nc.vector.tensor_scalar(V(pc), V(tmp), 0.0, 57.0,
                                    op0=ALU.max, op1=ALU.min)
            nc.vector.tensor_tensor(V(gA), V(pc), V(qlt), op=ALU.subtract)
            nc.vector.tensor_scalar(V(gA), V(gA), -1.0, 1.0,
                                    op0=ALU.mult, op1=ALU.add)
            nc.vector.tensor_tensor(V(hh), V(qrb), V(pc), op=ALU.subtract)
            nc.vector.tensor_scalar(V(hh), V(hh), -1.0, 1.0,
                                    op0=ALU.mult, op1=ALU.add)

        halfchain(Y, offT2)
        halfchain(X, offT2)
        # trunc factors (x axis): t0 = (gA_x >= 1), t1 = (hh_x >= 1)
        nc.vector.tensor_scalar(V9(t0), X(gA), 1.0, None, op0=ALU.is_ge)
        nc.vector.tensor_scalar(V9(t1), X(hh), 1.0, None, op0=ALU.is_ge)
        # weights
        nc.vector.tensor_tensor(V9(w_lt), Y(gA), V9(t0), op=ALU.mult)
        nc.vector.tensor_tensor(V9(w_rb), Y(hh), V9(t1), op=ALU.mult)
        nc.vector.tensor_tensor(V9(w_lb), Y(gA), X(hh), op=ALU.mult)
        nc.vector.tensor_tensor(V9(w_rt), Y(hh), X(gA), op=ALU.mult)
        # interior slot weights (anti-diagonal pair + exact-hit fold):
        #   s0 = w_lt + w_rt*(1-t0)   (slot0 = bl normally, lt at exact hit)
        #   s1 = w_lb + t0*(w_rt - w_lb)  (slot1 = tr normally, bl at hit)
        nc.vector.tensor_tensor(tt[:], t0[:], w_rt[:], op=ALU.mult)
        nc.vector.tensor_tensor(s0f[:], w_lt[:], w_rt[:], op=ALU.add)
        nc.vector.tensor_tensor(s0f[:], s0f[:], tt[:], op=ALU.subtract)
        nc.vector.tensor_tensor(s1f[:], w_rt[:], w_lb[:], op=ALU.subtract)
        nc.vector.tensor_tensor(s1f[:], s1f[:], t0[:], op=ALU.mult)
        nc.vector.tensor_tensor(s1f[:], s1f[:], w_lb[:], op=ALU.add)
        s0, s1 = s0f, s1f  # scalar operands must stay f32
        wltb, wrbb, wlbb, wrtb = w_lt, w_rb, w_lb, w_rt
        # gather base index: s = clip(q, -1, 57); e = sy*60 + sx + 61
        # idx rows (C-units) of xr2: edge tiles 2e; interior 2e + 1 - t0
        nc.vector.tensor_scalar(q[:], q[:], -1.0, 57.0, op0=ALU.max, op1=ALU.min)
        nc.vector.tensor_scalar(V9(idxf), Y(q), 120.0, 122.0,
                                op0=ALU.mult, op1=ALU.add)
        nc.vector.tensor_tensor(V9(idxf), V9(idxf), X(q), op=ALU.add)
        nc.vector.tensor_tensor(V9(idxf), V9(idxf), X(q), op=ALU.add)
        # interior tiles: idx += 1 - t0
        iv = idxf[:].rearrange("p (t k) -> p t k", k=9)[:, INT_T0:INT_T1, :]
        tv = t0[:].rearrange("p (t k) -> p t k", k=9)[:, INT_T0:INT_T1, :]
        nc.vector.tensor_scalar(iv, iv, 1.0, None, op0=ALU.add)
        nc.vector.tensor_tensor(iv, iv, tv, op=ALU.subtract)
        # int16 idx in per-tap layout [edge0 | edge23 | edge24 | interior 1..22]
        # (8 wrapped cols per tile) via wrap DMAs:
        #   interior: widx[r, n*200 + 24 + 8(t-1) + k] = idx16[16k + r, t*9 + n]
        idx16 = sb.tile([C, NTILE * NTAP], I16)
        nc.vector.tensor_copy(out=idx16[:], in_=idxf[:])
        # (n, t)-major splits: interior col n*22 + (t-1); edge col n*3 + ei
        idx16i = sb.tile([C, NTAP * 22], I16)
        idx16e = sb.tile([C, NTAP * 3], I16)
        i3 = idx16[:].rearrange("p (t n) -> p t n", t=NTILE)
        nc.vector.tensor_copy(
            out=idx16i[:].rearrange("p (n s) -> p s n", s=22),
            in_=i3[:, INT_T0:INT_T1, :])
        nc.vector.tensor_copy(
            out=idx16e[:].rearrange("p (n s) -> p s n", s=3)[:, 0:1, :],
            in_=i3[:, 0:1, :])
        nc.vector.tensor_copy(
            out=idx16e[:].rearrange("p (n s) -> p s n", s=3)[:, 1:3, :],
            in_=i3[:, 23:25, :])
        widx_i = sb.tile([C, NTAP * 176], I16)   # interior: n*176 + 8(t-1) + k
        widx_e = sb.tile([C, NTAP * 24], I16)    # edge: n*24 + 8*ei + k
        for k in range(8):
            dsti = widx_i[0:16, :].rearrange(
                "p (s k) -> p s k", k=8)[:, :, k:k + 1]
            dste = widx_e[0:16, :].rearrange(
                "p (s k) -> p s k", k=8)[:, :, k:k + 1]
            nc.sync.dma_start(dsti, idx16i[16 * k:16 * k + 16, :].rearrange(
                "p (s u) -> p s u", u=1))
            nc.sync.dma_start(dste, idx16e[16 * k:16 * k + 16, :].rearrange(
                "p (s u) -> p s u", u=1))
        for lo, hi in ((16, 32), (32, 64), (64, 128)):
            nc.sync.dma_start(widx_i[lo:hi, :], widx_i[0:hi - lo, :])
            nc.sync.dma_start(widx_e[lo:hi, :], widx_e[0:hi - lo, :])

        # keep the PE clock ramped through the DVE-heavy phase D: a few dummy
        # matmuls anchored on D outputs execute interspersed with D.
        for anchor in (w_lt, w_rb, w_lb, w_rt, s0f, s1f, idxf):
            nc.tensor.matmul(psw[:, 0:56], ident[:, 0:1], anchor[:, 0:56],
                             start=True, stop=True)

        # ---- E: per-tap gather + blend + transpose + conv ----
        psCctx.close()  # release phase-C PSUM banks
        psWctx.close()
        psT = ctx.enter_context(tc.tile_pool(name="psT", bufs=2, space="PSUM"))
        psO = ctx.enter_context(tc.tile_pool(name="psO", bufs=1, space="PSUM"))
        accs = [psO.tile([OUT, 512], F32, tag=f"acc{ch}", name=f"acc{ch}")
                for ch in range(6)]
        out6 = sb.tile([OUT, C], F32)  # SBUF accumulator for pixels 3072:3200

        # xr2 viewed as overlapping runs: row i = elements [i*C, i*C + len)
        xr_pair = bass.AP(tensor=d_xr2.tensor, offset=0,
                          ap=[[C, XR_ROWS - 1], [1, 2 * C]])
        xr_quad = bass.AP(tensor=d_xr2.tensor, offset=0,
                          ap=[[C, XR_ROWS - 3], [1, 4 * C]])
        for tap in range(NTAP):
            g2 = gpool.tile([C, NTILE, 2 * C], BF16, tag="g2", name="g2")
            g4 = gpool.tile([C, 4, 4 * C], BF16, tag="g4", name="g4")
            # edge tiles 0,23,24: 4-corner quad, 1 desc/(pix,tap)
            nc.gpsimd.dma_gather(
                out_ap=g4[:, 0:3, :], in_ap=xr_quad,
                idxs_ap=widx_e[:, tap * 24: tap * 24 + 24],
                num_idxs=384, num_idxs_reg=384,
                elem_size=4 * C, elem_step=C)
            # interior tiles 1..22: anti-diagonal pair, 1 desc/(pix,tap);
            # chunked to fit the default 1024-entry SWDGE ring
            for j0, nj in ((0, 1024), (1024, 1024), (2048, 768)):
                c0 = tap * 176 + j0 // 16
                nc.gpsimd.dma_gather(
                    out_ap=g2[:, INT_T0 + j0 // 128: INT_T0 + (j0 + nj) // 128, :],
                    in_ap=xr_pair,
                    idxs_ap=widx_i[:, c0: c0 + nj // 16],
                    num_idxs=nj, num_idxs_reg=nj,
                    elem_size=2 * C, elem_step=C)

            rhs = rpool.tile([C, NPAD], BF16, tag="rhs")
            for tq in range(7):  # quads of pixel-tiles
                ntq = 4 if tq < 6 else 1
                pst = psT.tile([C, 512], F32, tag="pstr")
                pstb = pst[:].bitcast(BF16)
                for k in range(ntq):
                    t = tq * 4 + k
                    wcol = slice(t * NTAP + tap, t * NTAP + tap + 1)
                    xo = xpool.tile([C, C], BF16, tag="xo")
                    ve = nc.vector
                    if t in EDGE_T:
                        ei = 0 if t == 0 else t - 22
                        ve.tensor_scalar(
                            xo[:], g4[:, ei, 0:C], wltb[:, wcol], None,
                            op0=ALU.mult)
                        ve.scalar_tensor_tensor(
                            xo[:], g4[:, ei, C:2 * C], wrtb[:, wcol], xo[:],
                            op0=ALU.mult, op1=ALU.add)
                        ve.scalar_tensor_tensor(
                            xo[:], g4[:, ei, 2 * C:3 * C], wlbb[:, wcol], xo[:],
                            op0=ALU.mult, op1=ALU.add)
                        ve.scalar_tensor_tensor(
                            xo[:], g4[:, ei, 3 * C:4 * C], wrbb[:, wcol], xo[:],
                            op0=ALU.mult, op1=ALU.add)
                    else:
                        ve.tensor_scalar(
                            xo[:], g2[:, t, 0:C], s0[:, wcol], None,
                            op0=ALU.mult)
                        ve.scalar_tensor_tensor(
                            xo[:], g2[:, t, C:2 * C], s1[:, wcol], xo[:],
                            op0=ALU.mult, op1=ALU.add)
                    nc.tensor.transpose(pstb[:, k * C:(k + 1) * C], xo[:],
                                        identb[:])
                nc.scalar.activation(rhs[:, tq * 512: tq * 512 + ntq * C],
                                     pstb[:, :ntq * C], ACTF.Copy)

            for ch in range(6):
                nc.tensor.matmul(
                    accs[ch][:],
                    convw[:, tap * OUT:(tap + 1) * OUT],
                    rhs[:, 512 * ch: 512 * ch + 512],
                    start=(tap == 0), stop=(tap == NTAP - 1))
            ps6 = psT.tile([C, 512], F32, tag="pstr")
            nc.tensor.matmul(ps6[:, :C],
                             convw[:, tap * OUT:(tap + 1) * OUT],
                             rhs[:, 3072:3200],
                             start=True, stop=True)
            if tap == 0:
                nc.vector.tensor_copy(out=out6[:], in_=ps6[:, :C])
            else:
                nc.vector.tensor_tensor(out6[:], out6[:], ps6[:, :C],
                                        op=ALU.add)

        # ---- F: output ----
        for ch in range(6):
            ob = opool.tile([OUT, 512], F32, tag="ob")
            nc.scalar.activation(ob[:], accs[ch][:], ACTF.Copy)
            nc.sync.dma_start(d_out[:, 512 * ch:512 * ch + 512], ob[:])
        nc.sync.dma_start(d_out[:, 3072:3200], out6[:])


# ---------------- host-side input prep ----------------

def prep_core_inputs(xb, offset_w, offset_b, conv_w):
    """Build the per-core in_map from one batch image [C, H, W] + weights."""
    f32 = np.float32
    xb = np.asarray(xb, f32)
    xp = np.pad(xb, ((0, 0), (1, 1), (1, 1)))                   # [C, 58, 58]
    xcp = np.ascontiguousarray(xp.reshape(C, HP * HP))
    xpm = np.ascontiguousarray(xp.transpose(1, 2, 0).reshape(HP * HP, C))
    xr60 = np.pad(xp, ((0, 0), (1, 1), (1, 1)), mode="edge")    # [C, 60, 60]
    xr60 = xr60.transpose(1, 2, 0)                              # [60, 60, C]
    # interleaved row pairs: xr2[2*(y*60+x)] = xr60[y,x]; [.. +1] = xr60[y+1,x]
    xr2 = np.stack([xr60[:-1], xr60[1:]], axis=2)               # [59, 60, 2, C]
    xr2 = np.ascontiguousarray(xr2.reshape(XR_ROWS, C)).astype(ml_dtypes.bfloat16)

    offw = np.empty((C, NTAP * 18), f32)
    convw = np.empty((C, NTAP * OUT), f32)
    for tap in range(NTAP):
        ky, kx = tap // 3, tap % 3
        offw[:, tap * 18:(tap + 1) * 18] = np.asarray(offset_w, f32)[:, :, ky, kx].T
        convw[:, tap * OUT:(tap + 1) * OUT] = np.asarray(conv_w, f32)[:, :, ky, kx].T
    offb = np.asarray(offset_b, f32).reshape(18, 1)

    # base grid [128, 25*18]: partition p, col t*18+k -> pixel t*128+p (col-major)
    r = np.arange(-1, 2, dtype=f32)
    py_n, px_n = np.meshgrid(r, r, indexing="ij")
    pny, pnx = py_n.ravel(), px_n.ravel()
    gy = np.arange(1, 57, dtype=f32)
    p0y, p0x = np.meshgrid(gy, gy, indexing="ij")
    p0yc, p0xc = p0y.T.ravel(), p0x.T.ravel()      # col-major pixels
    base = np.empty((NPAD, 18), f32)
    base[:NPIX, :NTAP] = p0yc[:, None] + pny[None, :]
    base[:NPIX, NTAP:] = p0xc[:, None] + pnx[None, :]
    base[NPIX:, :NTAP] = 28.0 + pny[None, :]
    base[NPIX:, NTAP:] = 28.0 + pnx[None, :]
    base = np.ascontiguousarray(
        base.reshape(NTILE, C, 18).transpose(1, 0, 2).reshape(C, NTILE * 18))

    tri = np.triu(np.ones((C, C), f32), 1)  # tri[p, m] = 1 iff p < m
    pixid1 = np.ascontiguousarray(
        (np.arange(NTILE)[None, :] * C + np.arange(C)[:, None] + 1)
        .astype(f32))
    return {"xcp": xcp, "xpm": xpm, "xr2": xr2, "offw": offw, "offb": offb,
            "convw": convw.astype(ml_dtypes.bfloat16), "base": base,
            "tri": tri, "pixid1": pixid1}


def postprocess(out_np):
    """[OUT, 3200] col-major -> [OUT, 56, 56]."""
    o = out_np[:, :NPIX].reshape(OUT, W, H).transpose(0, 2, 1)
    return np.ascontiguousarray(o)


# ---------------- entry point ----------------

N_CORES = 8
_cache = {}


def _build():
    if "nc" in _cache:
        return _cache["nc"]
    nc = bacc.Bacc("TRN2", target_bir_lowering=False, debug=False,
                   enable_asserts=True, num_devices=N_CORES)
    build_kernel(nc)
    nc.compile()
    nc.m = get_hw_module(nc.m)
    _cache["nc"] = nc
    return nc


def kernel(x, offset_w, offset_b, conv_w):
    x = np.asarray(x, np.float32)
    assert x.shape == (N_CORES, C, H, W), x.shape
    nc = _build()
    in_maps = [prep_core_inputs(x[b], offset_w, offset_b, conv_w)
               for b in range(N_CORES)]
    res = run_bass_kernel_spmd(nc, in_maps, core_ids=list(range(N_CORES)))
    outs = [postprocess(res.results[b]["out"]) for b in range(N_CORES)]
    return np.stack(outs).astype(np.float32)



# revision 32
# speedup vs baseline: 1.2837x; 1.0051x over previous
"""Deformable-conv kernel for Trainium2: 8-core data-parallel over batch.

kernel(x, offset_w, offset_b, conv_w) -> [8, 128, 56, 56] float32.
Each NeuronCore processes one batch image:
  offset conv in true-F32 PE matmuls (the reference sampler is discontinuous
  at integer x-coords, so offsets need ~1e-7 accuracy to reproduce its
  floor/trunc decisions) -> pixel-partitioned offsets (PE transpose)
  -> index/bilinear-weight math (DVE) -> bf16 indirect-DMA gathers from a
  row-pair-interleaved padded map (interior pixels: one 512B descriptor per
  (pixel, tap) fetching the anti-diagonal [bot-left, top-right] corner pair,
  index shifted by the exact-integer-hit mask; edge tiles: one 1KB 4-corner
  descriptor) -> 2-term (interior) / 4-term (edge) blend (DVE) -> bf16 PE
  transpose -> 3x3/stride-3 conv as 9 accumulating bf16 matmuls (PSUM).
"""
import sys
for _p in ("/opt/trn_rl_repo", "/root/.axon_site/_ro/trn_rl_repo"):
    if _p not in sys.path:
        sys.path.append(_p)

from contextlib import ExitStack

import numpy as np
import ml_dtypes

import concourse.bass as bass
import concourse.bacc as bacc
import concourse.mybir as mybir
import concourse.tile as tile
from concourse.masks import make_identity
from concourse.bass_utils import run_bass_kernel_spmd
from concourse.bass_interp import get_hw_module

F32 = mybir.dt.float32
BF16 = mybir.dt.bfloat16
I32 = mybir.dt.int32
I16 = mybir.dt.int16
ALU = mybir.AluOpType
ACTF = mybir.ActivationFunctionType

DEBUG_FIX = False
USE_FIXUP = False
USE_F32R = False
FIXLEVEL = 5  # bisect knob: 1=flag+zero+dram rt, 2=+rank/table, 3=+patch gather,
              # 4=+precise conv, 5=full (delta scatter-add)
POOL_TAP = 9  # taps >= this run blends on GPSIMD instead of DVE (9 = never:
              # measured slower at 7/8 — GPSIMD overhead + gather-launch
              # serialization outweigh the DVE relief)
H = W = 56
HP = 58
NPIX = H * W          # 3136
NPAD = 3200           # padded pixel count (25 tiles of 128)
NTILE = 25
NTAP = 9
C = 128
OUT = 128
XR_ROWS = 3540 * 2    # interleaved row-pair map: entry e -> rows 2e, 2e+1
# edge tiles: pixel cols j<=2 or j>=52 live here (clip/trunc can fire in x)
EDGE_T = (0, 23, 24)
INT_T0, INT_T1 = 1, 23  # interior tiles [1, 23)


def build_kernel(nc):
    d = {
        "xcp": nc.dram_tensor("xcp", [C, HP * HP], F32, kind="ExternalInput").ap(),
        "xpm": nc.dram_tensor("xpm", [HP * HP, C], F32, kind="ExternalInput").ap(),
        "xr2": nc.dram_tensor("xr2", [XR_ROWS, C], BF16, kind="ExternalInput").ap(),
        "offw": nc.dram_tensor("offw", [C, NTAP * 18], F32, kind="ExternalInput").ap(),
        "offb": nc.dram_tensor("offb", [18, 1], F32, kind="ExternalInput").ap(),
        "convw": nc.dram_tensor("convw", [C, NTAP * OUT], BF16, kind="ExternalInput").ap(),
        "base": nc.dram_tensor("base", [C, NTILE * 18], F32, kind="ExternalInput").ap(),
        "tri": nc.dram_tensor("tri", [C, C], F32, kind="ExternalInput").ap(),
        "pixid1": nc.dram_tensor("pixid1", [C, NTILE], F32, kind="ExternalInput").ap(),
        "out": nc.dram_tensor("out", [OUT, NPAD], F32, kind="ExternalOutput").ap(),
    }
    if DEBUG_FIX:
        for nm, shp, dt in (("dbg_offTm", [C, NTILE * 18], F32),
                            ("dbg_offT2", [C, NTILE * 18], F32),
                            ("dbg_fl", [C, NTILE], F32),
                            ("dbg_rank", [C, NTILE], F32),
                            ("dbg_tb", [16, 16], F32),
                            ("dbg_delta", [C, 2 * 18], F32),
                            ("dbg_prec", [18, 256], F32),
                            ("dbg_gk", [C, 3 * 2 * 3 * C], F32)):
            d[nm] = nc.dram_tensor(nm, shp, dt, kind="ExternalOutput").ap()
    with tile.TileContext(nc) as tc:
        emit(tc, d)
    return nc


def emit(tc, d):
    d_xcp, d_xpm, d_xr2 = d["xcp"], d["xpm"], d["xr2"]
    d_offw, d_offb, d_convw = d["offw"], d["offb"], d["convw"]
    d_base, d_tri, d_pixid1, d_out = d["base"], d["tri"], d["pixid1"], d["out"]
    nc = tc.nc
    F32R = mybir.dt.float32r
    ctx = ExitStack()
    with ctx:
        consts = ctx.enter_context(tc.tile_pool(name="consts", bufs=1))
        sb = ctx.enter_context(tc.tile_pool(name="sb", bufs=1))
        gpool = ctx.enter_context(tc.tile_pool(name="gpool", bufs=3))
        xpool = ctx.enter_context(tc.tile_pool(name="xpool", bufs=8))
        rpool = ctx.enter_context(tc.tile_pool(name="rpool", bufs=2))
        opool = ctx.enter_context(tc.tile_pool(name="opool", bufs=2))
        dpool = ctx.enter_context(tc.tile_pool(name="dpool", bufs=1, space="DRAM"))
        psWctx = ExitStack()
        psW = psWctx.enter_context(tc.tile_pool(name="psW", bufs=1,
                                                space="PSUM"))
        psBctx = ExitStack()
        psB = psBctx.enter_context(tc.tile_pool(name="psB", bufs=1, space="PSUM"))

        # ---- A: loads ----
        xcpr = None
        if USE_FIXUP or USE_F32R:
            xcpr = consts.tile([C, HP * HP], F32R)
            nc.gpsimd.dma_start(xcpr[:], d_xcp[:])
        xcpf = consts.tile([C, HP * HP], F32)
        nc.sync.dma_start(xcpf[:], d_xcp[:])
        offw = consts.tile([C, NTAP * 18], F32)
        nc.sync.dma_start(offw[:], d_offw[:])
        offwr = None
        if USE_FIXUP or USE_F32R:
            offwr = consts.tile([C, NTAP * 18], F32R)
            nc.gpsimd.dma_start(offwr[:], d_offw[:])
        convw = consts.tile([C, NTAP * OUT], BF16)
        nc.sync.dma_start(convw[:], d_convw[:])
        offb = consts.tile([18, 1], F32)
        nc.sync.dma_start(offb[:], d_offb[:])
        base = consts.tile([C, NTILE * 18], F32)
        nc.sync.dma_start(base[:], d_base[:])
        if USE_FIXUP:
            tri = consts.tile([C, C], F32)
            nc.sync.dma_start(tri[:], d_tri[:])
            pixid1 = consts.tile([C, NTILE], F32)
            nc.sync.dma_start(pixid1[:], d_pixid1[:])
        ident = consts.tile([C, C], F32)
        make_identity(nc, ident[:])
        identb = consts.tile([C, C], BF16)
        nc.vector.tensor_copy(out=identb[:], in_=ident[:])

        # PE p-state warmup: ~70 cheap matmuls fill the PE queue for >3us of
        # busy time (hidden under the input DMA loads), so phase B's matmuls
        # dispatch with the tensor clock fully ramped (cost-model p-state).
        wz = consts.tile([C, 64], BF16)
        nc.vector.memset(wz[:], 0.0)
        psw = psW.tile([1, 64], F32, tag="psw", name="psw")
        for _ in range(70):
            nc.tensor.matmul(psw[:], identb[:, 0:1], wz[:],
                             start=True, stop=True)

        # ---- B: offset conv, fast F32R pass (fixed up below for pixels whose
        # x-offset lands near an integer, where the reference's trunc/floor
        # decisions are discontinuous) ----
        # col-major output pixels: chunk c covers j in [8c, 8c+8), all i.
        off_sb = sb.tile([18, NPAD], F32)
        xcp3 = (xcpr if (USE_FIXUP or USE_F32R) else xcpf)[:].rearrange(
            "p (y x) -> p y x", y=HP)
        pss = [psB.tile([18, 448], F32, tag=f"psB{ch}", name=f"psB{ch}")
               for ch in range(7)]
        for tap in range(NTAP):
            ky, kx = tap // 3, tap % 3
            for ch in range(7):
                rhs = xcp3[:, ky:ky + 56, kx + 8 * ch: kx + 8 * ch + 8] \
                    .transpose([0, 2, 1])
                lhsw = offwr if (USE_FIXUP or USE_F32R) else offw
                nc.tensor.matmul(
                    pss[ch][:], lhsw[:, tap * 18:(tap + 1) * 18], rhs,
                    start=(tap == 0), stop=(tap == NTAP - 1))
        for ch in range(7):
            nc.scalar.activation(off_sb[:, 448 * ch:448 * (ch + 1)], pss[ch][:],
                                 ACTF.Identity, bias=offb[:, :1], scale=1.0)
        # pad pixels: 0.5 keeps them far from the near-integer flag band
        nc.vector.memset(off_sb[:, NPIX:], 0.5)

        # ---- C: transpose offsets to pixel-partitioned ----
        psBctx.close()
        psCctx = ExitStack()
        psC = psCctx.enter_context(tc.tile_pool(name="psC", bufs=2, space="PSUM"))
        offT = sb.tile([C, NTILE * 18], F32)
        for t in range(NTILE):
            pst = psC.tile([C, 18], F32, tag="psC")
            nc.tensor.transpose(pst[:], off_sb[:, t * C:(t + 1) * C],
                                ident[:18, :18])
            nc.scalar.activation(offT[:, t * 18:(t + 1) * 18], pst[:],
                                 ACTF.Copy)

        # ---- B2: precise fixup of near-integer x-offsets ----
        # Flag pixels with any x-offset within TH of an integer, zero their
        # x-offsets, round-trip offsets through DRAM (64-f32-padded pixel
        # rows), recompute flagged pixels' offsets with true-F32 matmuls on
        # gathered patches, and scatter-ADD them into the zeroed rows.
        def Y(ap):  # y-axis slice of [128, 25*18] -> [128, 25, 9]
            return ap[:].rearrange("p (t k) -> p t k", k=18)[:, :, 0:9]

        def X(ap):
            return ap[:].rearrange("p (t k) -> p t k", k=18)[:, :, 9:18]

        def V9(ap):  # [128, 25*9] -> [128, 25, 9]
            return ap[:].rearrange("p (t k) -> p t k", k=9)

        if USE_FIXUP:
            # Both x- AND y-offsets near an integer make the reference's
            # corner/trunc decisions discontinuous (only the anti-diagonal
            # corner pair survives generically, and which rows it sits on
            # flips at every y-integer crossing), so flag BOTH halves and
            # recompute flagged pixels' full 18-offset vector exactly.
            TH = 1e-3
            NSLOT = 256  # flagged-pixel capacity (E[flags] ~ 134/image)
            psF = psCctx.enter_context(tc.tile_pool(name="psF", bufs=1, space="PSUM"))
            d_offd = dpool.tile([4096, 64], F32, name="d_offd")
            d_ftab = dpool.tile([512, 64], F32, name="d_ftab")

            def Y(ap):  # y-axis slice of [128, 25*18] -> [128, 25, 9]
                return ap[:].rearrange("p (t k) -> p t k", k=18)[:, :, 0:9]

            def X(ap):
                return ap[:].rearrange("p (t k) -> p t k", k=18)[:, :, 9:18]

            def V18(ap):  # [128, 25*18] -> [128, 25, 18]
                return ap[:].rearrange("p (t k) -> p t k", k=18)

            fr = sb.tile([C, NTILE * 18], F32, tag="fxfr")
            fri = sb.tile([C, NTILE * 18], I32, tag="fxfri")
            fl2 = sb.tile([C, NTILE * 18], F32, tag="fxfl2")
            fl = sb.tile([C, NTILE], F32)
            rank = sb.tile([C, NTILE], F32)
            svec = sb.tile([C, 1], F32, tag="fxs")
            pbase = sb.tile([C, 1], F32)
            # fr = frac(off) via exact floor; near-integer iff fr<TH or fr>1-TH
            nc.vector.tensor_copy(out=V18(fri), in_=V18(offT))
            nc.vector.tensor_copy(out=V18(fr), in_=V18(fri))
            nc.vector.tensor_tensor(V18(fl2), V18(offT), V18(fr), op=ALU.is_lt)
            nc.vector.tensor_tensor(V18(fr), V18(fr), V18(fl2), op=ALU.subtract)
            nc.vector.tensor_tensor(V18(fr), V18(offT), V18(fr), op=ALU.subtract)
            nc.vector.tensor_scalar(V18(fl2), V18(fr), TH, None, op0=ALU.is_lt)
            nc.vector.tensor_scalar(V18(fr), V18(fr), 1.0 - TH, None, op0=ALU.is_gt)
            nc.vector.tensor_tensor(V18(fl2), V18(fl2), V18(fr), op=ALU.max)
            nc.vector.tensor_reduce(
                out=fl[:].rearrange("p (t u) -> p t u", u=1),
                in_=V18(fl2), axis=mybir.AxisListType.X, op=ALU.max)
            # zero flagged pixels' offsets (all 18) in place
            flb = fl2  # reuse
            nc.vector.tensor_scalar(flb[:, :NTILE], fl[:], -1.0, 1.0,
                                    op0=ALU.mult, op1=ALU.add)
            flbv = flb[:, :NTILE].rearrange("p (t u) -> p t u", u=1)
            flbb = bass.AP(tensor=flbv.tensor, offset=flbv.offset,
                           ap=[list(flbv.ap[0]), list(flbv.ap[1]), [0, 18]])
            nc.vector.tensor_tensor(V18(offT), V18(offT), flbb, op=ALU.mult)
            # offsets -> DRAM pixel rows (row = t*128+p, 64-f32 stride)
            od_w = d_offd[:].rearrange("(t p) c -> p t c", p=C)[:, :NTILE, 0:18]
            nc.sync.dma_start(od_w, offT[:])
            # ranks: pbase[p] = sum of flags on partitions < p; + exclusive scan
            if FIXLEVEL >= 2:
                nc.vector.tensor_reduce(out=svec[:], in_=fl[:],
                                        axis=mybir.AxisListType.X, op=ALU.add)
                psL = psF.tile([C, 1], F32, tag="psL")
                nc.tensor.matmul(psL[:], tri[:], svec[:], start=True, stop=True)
                nc.scalar.activation(pbase[:], psL[:], ACTF.Copy)
                nc.vector.tensor_tensor_scan(rank[:], fl[:], fl[:], initial=0.0,
                                             op0=ALU.add, op1=ALU.max)
                nc.vector.tensor_scalar(rank[:], rank[:], pbase[:, :1], None,
                                        op0=ALU.add)
                nc.vector.tensor_tensor(rank[:], rank[:], fl[:], op=ALU.subtract)
                nc.vector.tensor_scalar(rank[:], rank[:], NSLOT - 1.0, None,
                                        op0=ALU.min)
                # unflagged pixels -> dump slot NSLOT: every live slot gets
                # exactly one add (concurrent adds to one address lose updates)
                nc.vector.tensor_scalar(rank[:], rank[:], -float(NSLOT), None,
                                        op0=ALU.add)
                nc.vector.tensor_tensor(rank[:], rank[:], fl[:], op=ALU.mult)
                nc.vector.tensor_scalar(rank[:], rank[:], float(NSLOT), None,
                                        op0=ALU.add)
                # scatter fl*(pixid+1) into the NSLOT-slot table at rank
                vtab = sb.tile([C, NTILE], F32, tag="fxv")
                nc.vector.tensor_tensor(vtab[:], fl[:], pixid1[:], op=ALU.mult)
                rank16 = sb.tile([C, NTILE], I16)
                nc.vector.tensor_copy(out=rank16[:], in_=rank[:])
                rwr = sb.tile([C, 200], I16)
                for k in range(8):
                    dstr = rwr[0:16, :].rearrange(
                        "p (t k) -> p t k", k=8)[:, :, k:k + 1]
                    nc.sync.dma_start(dstr, rank16[16 * k:16 * k + 16, :].rearrange(
                        "p (t u) -> p t u", u=1))
                for lo, hi in ((16, 32), (32, 64), (64, 128)):
                    nc.sync.dma_start(rwr[lo:hi, :], rwr[0:hi - lo, :])
                zt = sb.tile([C, 1], F32, tag="fxz")
                nc.vector.memset(zt[:], 0.0)
                ft_head = d_ftab[:, 0:1]
                nc.sync.dma_start(d_ftab[0:C, 0:1], zt[:])
                nc.sync.dma_start(d_ftab[C:2 * C, 0:1], zt[:])
                nc.gpsimd.dma_scatter_add(
                    out_ap=ft_head,
                    in_ap=vtab[:].rearrange("p (a u) -> p a u", u=1),
                    idxs_ap=rwr[:, :],
                    num_idxs=NTILE * C, num_idxs_reg=NTILE * C,
                    elem_size=1, elem_step=64)
            if FIXLEVEL >= 3:
                # read table (wrapped 16x16), derive patch-run + scatter indices
                tb = sb.tile([16, 16], F32)
                tb_src = d_ftab[0:NSLOT, :].rearrange(
                    "(c r) u -> r c u", r=16)[:, :, 0:1]
                nc.sync.dma_start(tb[:], tb_src)
                pixv = sb.tile([16, 16], F32, tag="fxp")
                jj = sb.tile([16, 16], F32, tag="fxj")
                ji = sb.tile([16, 16], I32, tag="fxji")
                sc16 = sb.tile([C, 16], I16)
                nc.vector.tensor_scalar(pixv[:], tb[:], 3200.0, -1.0,
                                        op0=ALU.min, op1=ALU.add)
                nc.vector.tensor_copy(out=sc16[0:16, :], in_=pixv[:])  # -1 pads
                for lo, hi in ((16, 32), (32, 64), (64, 128)):
                    nc.sync.dma_start(sc16[lo:hi, :], sc16[0:hi - lo, :])
                nc.vector.tensor_scalar(pixv[:], pixv[:], 0.0, None, op0=ALU.max)
                # i32 copy rounds-to-nearest; bias by -0.5+eps so round == floor
                nc.vector.tensor_scalar(jj[:], pixv[:], 1.0 / 56, 1e-4 - 0.5,
                                        op0=ALU.mult, op1=ALU.add)
                nc.vector.tensor_copy(out=ji[:], in_=jj[:])
                nc.vector.tensor_copy(out=jj[:], in_=ji[:])
                # rbase = 58*i + j = 58*pix - 3247*j  (i = pix - 56*j)
                nc.vector.tensor_scalar(jj[:], jj[:], -3247.0, None, op0=ALU.mult)
                nc.vector.tensor_scalar(pixv[:], pixv[:], 58.0, None, op0=ALU.mult)
                nc.vector.tensor_tensor(pixv[:], pixv[:], jj[:], op=ALU.add)
                pidxf = sb.tile([16, 48], F32, tag="fxpi")
                for ky in range(3):
                    nc.vector.tensor_scalar(pidxf[:, ky * 16:(ky + 1) * 16],
                                            pixv[:], 58.0 * ky, None, op0=ALU.add)
                pidx = sb.tile([C, 48], I16)
                nc.vector.tensor_copy(out=pidx[0:16, :], in_=pidxf[:])
                for lo, hi in ((16, 32), (32, 64), (64, 128)):
                    nc.sync.dma_start(pidx[lo:hi, :], pidx[0:hi - lo, :])
                # gather 3x3-row patches (3 one-row-triple runs per flagged pixel)
                gk = sb.tile([C, 3, 2, 3 * C], F32)
                xpm_runs = bass.AP(tensor=d_xpm.tensor, offset=0,
                                   ap=[[C, HP * HP - 2], [1, 3 * C]])
                for ky in range(3):
                    nc.gpsimd.dma_gather(
                        out_ap=gk[:, ky, :, :], in_ap=xpm_runs,
                        idxs_ap=pidx[:, ky * 16:(ky + 1) * 16],
                        num_idxs=NSLOT, num_idxs_reg=NSLOT,
                        elem_size=3 * C, elem_step=C)
            if FIXLEVEL >= 4:
                # transpose patches to channel-major, precise F32 conv, add bias
                patchf = sb.tile([C, NTAP * NSLOT], F32)
                for tap in range(NTAP):
                    ky, kx = tap // 3, tap % 3
                    for ch in range(2):
                        psK = psF.tile([C, C], F32, tag="psK")
                        nc.tensor.transpose(
                            psK[:], gk[:, ky, ch, kx * C:(kx + 1) * C], ident[:])
                        nc.scalar.activation(
                            patchf[:, tap * NSLOT + ch * C:
                                   tap * NSLOT + (ch + 1) * C],
                            psK[:], ACTF.Copy)
                psP = psF.tile([18, NSLOT], F32, tag="psP")
                for tap in range(NTAP):
                    nc.tensor.matmul(psP[:], offw[:, tap * 18:(tap + 1) * 18],
                                     patchf[:, tap * NSLOT:(tap + 1) * NSLOT],
                                     start=(tap == 0), stop=(tap == NTAP - 1))
                prec = sb.tile([18, NSLOT], F32)
                nc.scalar.activation(prec[:], psP[:], ACTF.Identity,
                                     bias=offb[:, :1], scale=1.0)
                delta = sb.tile([C, 2, 18], F32)
                for ch in range(2):
                    psQ = psF.tile([C, 18], F32, tag="psQ")
                    nc.tensor.transpose(psQ[:], prec[:, ch * C:(ch + 1) * C],
                                        ident[:18, :18])
                    nc.scalar.activation(delta[:, ch, :], psQ[:], ACTF.Copy)
            if FIXLEVEL >= 5:
                # scatter full 18-offset rows into the zeroed DRAM rows
                # (idx < 0 at the end ignored)
                od_x = d_offd[:, 0:18]
                nc.gpsimd.dma_scatter_add(
                    out_ap=od_x,
                    in_ap=delta[:, :, :],
                    idxs_ap=sc16[:, :], num_idxs=NSLOT, num_idxs_reg=NSLOT,
                    elem_size=18, elem_step=64)
            # corrected offsets back to SBUF
            offT2 = sb.tile([C, NTILE * 18], F32)
            nc.sync.dma_start(offT2[:], od_w)
            if DEBUG_FIX:
                nc.sync.dma_start(d["dbg_offTm"][:], offT[:])
                nc.sync.dma_start(d["dbg_offT2"][:], offT2[:])
                nc.sync.dma_start(d["dbg_fl"][:], fl[:])
                nc.sync.dma_start(d["dbg_rank"][:], rank[:])
                nc.sync.dma_start(d["dbg_tb"][:], tb[:])
                nc.sync.dma_start(d["dbg_delta"][:],
                                  delta[:].rearrange("p a b -> p (a b)"))
                nc.sync.dma_start(d["dbg_prec"][:], prec[:])
                nc.sync.dma_start(d["dbg_gk"][:],
                                  gk[:].rearrange("p a b c -> p (a b c)"))

        else:
            offT2 = offT

        # ---- D: index + weight math ----
        # layout [128, 25*18]: col (t*18 + k), k in 0..8 = y taps, 9..17 = x taps
        w_lt = sb.tile([C, NTILE * NTAP], F32)
        w_rb = sb.tile([C, NTILE * NTAP], F32)
        w_lb = sb.tile([C, NTILE * NTAP], F32)
        w_rt = sb.tile([C, NTILE * NTAP], F32)
        s0f = sb.tile([C, NTILE * NTAP], F32)
        s1f = sb.tile([C, NTILE * NTAP], F32)
        idxf = sb.tile([C, NTILE * NTAP], F32)

        tmp = sb.tile([C, NTILE * 18], F32, tag="dtmp")      # p
        q = sb.tile([C, NTILE * 18], F32, tag="dtmp2")       # q = floor(p)
        qlt = sb.tile([C, NTILE * 18], F32, tag="dtmp3")
        qrb = sb.tile([C, NTILE * 18], F32, tag="dtmp4")
        pc = sb.tile([C, NTILE * 18], F32, tag="dtmp5")
        gA = sb.tile([C, NTILE * 18], F32, tag="dtmp6")      # 1 - f
        hh = sb.tile([C, NTILE * 18], F32, tag="dtmp7")      # 1 - (qrb - pc)
        t0 = sb.tile([C, NTILE * NTAP], F32, tag="dtmp8")
        t1 = sb.tile([C, NTILE * NTAP], F32, tag="dtmp9")
        tt = sb.tile([C, NTILE * NTAP], F32, tag="dtmp10")

        # per-half chain: y-half runs on the fast offsets (untouched by the
        # fixup) and overlaps the fixup's DMA latency; x-half waits for offT2
        ti = sb.tile([C, NTILE * 18], I32, tag="dti")

        def halfchain(V, src):
            nc.vector.tensor_tensor(V(tmp), V(base), V(src), op=ALU.add)
            nc.vector.tensor_copy(out=V(ti), in_=V(tmp))
            nc.vector.tensor_copy(out=V(q), in_=V(ti))
            nc.vector.tensor_tensor(V(qlt), V(tmp), V(q), op=ALU.is_lt)
            nc.vector.tensor_tensor(V(q), V(q), V(qlt), op=ALU.subtract)
            nc.vector.tensor_scalar(V(qlt), V(q), 0.0, 57.0,
                                    op0=ALU.max, op1=ALU.min)
            nc.vector.tensor_scalar(V(qrb), V(q), 1.0, 0.0,
                                    op0=ALU.add, op1=ALU.max)
            nc.vector.tensor_scalar(V(qrb), V(qrb), 57.0, None, op0=ALU.min)
            nc.vector.tensor_scalar(V(pc), V(tmp), 0.0, 57.0,
                                    op0=ALU.max, op1=ALU.min)
            nc.vector.tensor_tensor(V(gA), V(pc), V(qlt), op=ALU.subtract)
            nc.vector.tensor_scalar(V(gA), V(gA), -1.0, 1.0,
                                    op0=ALU.mult, op1=ALU.add)
            nc.vector.tensor_tensor(V(hh), V(qrb), V(pc), op=ALU.subtract)
            nc.vector.tensor_scalar(V(hh), V(hh), -1.0, 1.0,
                                    op0=ALU.mult, op1=ALU.add)

        halfchain(Y, offT2)
        halfchain(X, offT2)
        # trunc factors (x axis): t0 = (gA_x >= 1), t1 = (hh_x >= 1)
        nc.vector.tensor_scalar(V9(t0), X(gA), 1.0, None, op0=ALU.is_ge)
        nc.vector.tensor_scalar(V9(t1), X(hh), 1.0, None, op0=ALU.is_ge)
        # weights
        nc.vector.tensor_tensor(V9(w_lt), Y(gA), V9(t0), op=ALU.mult)
        nc.vector.tensor_tensor(V9(w_rb), Y(hh), V9(t1), op=ALU.mult)
        nc.vector.tensor_tensor(V9(w_lb), Y(gA), X(hh), op=ALU.mult)
        nc.vector.tensor_tensor(V9(w_rt), Y(hh), X(gA), op=ALU.mult)
        # interior slot weights (anti-diagonal pair + exact-hit fold):
        #   s0 = w_lt + w_rt*(1-t0)   (slot0 = bl normally, lt at exact hit)
        #   s1 = w_lb + t0*(w_rt - w_lb)  (slot1 = tr normally, bl at hit)
        nc.vector.tensor_tensor(tt[:], t0[:], w_rt[:], op=ALU.mult)
        nc.vector.tensor_tensor(s0f[:], w_lt[:], w_rt[:], op=ALU.add)
        nc.vector.tensor_tensor(s0f[:], s0f[:], tt[:], op=ALU.subtract)
        nc.vector.tensor_tensor(s1f[:], w_rt[:], w_lb[:], op=ALU.subtract)
        nc.vector.tensor_tensor(s1f[:], s1f[:], t0[:], op=ALU.mult)
        nc.vector.tensor_tensor(s1f[:], s1f[:], w_lb[:], op=ALU.add)
        s0, s1 = s0f, s1f  # scalar operands must stay f32
        wltb, wrbb, wlbb, wrtb = w_lt, w_rb, w_lb, w_rt
        # gather base index: s = clip(q, -1, 57); e = sy*60 + sx + 61
        # idx rows (C-units) of xr2: edge tiles 2e; interior 2e + 1 - t0
        nc.vector.tensor_scalar(q[:], q[:], -1.0, 57.0, op0=ALU.max, op1=ALU.min)
        nc.vector.tensor_scalar(V9(idxf), Y(q), 120.0, 122.0,
                                op0=ALU.mult, op1=ALU.add)
        nc.vector.tensor_tensor(V9(idxf), V9(idxf), X(q), op=ALU.add)
        nc.vector.tensor_tensor(V9(idxf), V9(idxf), X(q), op=ALU.add)
        # interior tiles: idx += 1 - t0
        iv = idxf[:].rearrange("p (t k) -> p t k", k=9)[:, INT_T0:INT_T1, :]
        tv = t0[:].rearrange("p (t k) -> p t k", k=9)[:, INT_T0:INT_T1, :]
        nc.vector.tensor_scalar(iv, iv, 1.0, None, op0=ALU.add)
        nc.vector.tensor_tensor(iv, iv, tv, op=ALU.subtract)
        # int16 idx in per-tap layout [edge0 | edge23 | edge24 | interior 1..22]
        # (8 wrapped cols per tile) via wrap DMAs:
        #   interior: widx[r, n*200 + 24 + 8(t-1) + k] = idx16[16k + r, t*9 + n]
        idx16 = sb.tile([C, NTILE * NTAP], I16)
        nc.vector.tensor_copy(out=idx16[:], in_=idxf[:])
        # (n, t)-major splits: interior col n*22 + (t-1); edge col n*3 + ei
        idx16e = sb.tile([C, NTAP * 3], I16)
        i3 = idx16[:].rearrange("p (t n) -> p t n", t=NTILE)
        # taps split into half A (0..3) and half B (4..8) with SEPARATE wrap
        # tiles, so half A's gathers launch while half B's wraps still run.
        NTA, NTB = 4, 5
        idx16iA = sb.tile([C, NTA * 22], I16)
        idx16iB = sb.tile([C, NTB * 22], I16)
        nc.vector.tensor_copy(
            out=idx16iA[:].rearrange("p (n s) -> p s n", s=22),
            in_=i3[:, INT_T0:INT_T1, 0:NTA])
        nc.vector.tensor_copy(
            out=idx16iB[:].rearrange("p (n s) -> p s n", s=22),
            in_=i3[:, INT_T0:INT_T1, NTA:NTAP])
        nc.vector.tensor_copy(
            out=idx16e[:].rearrange("p (n s) -> p s n", s=3)[:, 0:1, :],
            in_=i3[:, 0:1, :])
        nc.vector.tensor_copy(
            out=idx16e[:].rearrange("p (n s) -> p s n", s=3)[:, 1:3, :],
            in_=i3[:, 23:25, :])
        widx_iA = sb.tile([C, NTA * 176], I16)  # interior: n*176 + 8(t-1) + k
        widx_iB = sb.tile([C, NTB * 176], I16)
        widx_e = sb.tile([C, NTAP * 24], I16)   # edge: n*24 + 8*ei + k
        for k in range(8):
            dsti = widx_iA[0:16, :].rearrange(
                "p (s k) -> p s k", k=8)[:, :, k:k + 1]
            dste = widx_e[0:16, :].rearrange(
                "p (s k) -> p s k", k=8)[:, :, k:k + 1]
            nc.sync.dma_start(dsti, idx16iA[16 * k:16 * k + 16, :].rearrange(
                "p (s u) -> p s u", u=1))
            nc.sync.dma_start(dste, idx16e[16 * k:16 * k + 16, :].rearrange(
                "p (s u) -> p s u", u=1))
        for lo, hi in ((16, 32), (32, 64), (64, 128)):
            nc.sync.dma_start(widx_iA[lo:hi, :], widx_iA[0:hi - lo, :])
            nc.sync.dma_start(widx_e[lo:hi, :], widx_e[0:hi - lo, :])
        for k in range(8):
            dsti = widx_iB[0:16, :].rearrange(
                "p (s k) -> p s k", k=8)[:, :, k:k + 1]
            nc.sync.dma_start(dsti, idx16iB[16 * k:16 * k + 16, :].rearrange(
                "p (s u) -> p s u", u=1))
        for lo, hi in ((16, 32), (32, 64), (64, 128)):
            nc.sync.dma_start(widx_iB[lo:hi, :], widx_iB[0:hi - lo, :])

        # keep the PE clock ramped through the DVE-heavy phase D: a few dummy
        # matmuls anchored on D outputs execute interspersed with D.
        for anchor in (w_lt, w_rb, w_lb, w_rt, s0f, s1f, idxf):
            nc.tensor.matmul(psw[:, 0:56], ident[:, 0:1], anchor[:, 0:56],
                             start=True, stop=True)

        # ---- E: per-tap gather + blend + transpose + conv ----
        psCctx.close()  # release phase-C PSUM banks
        psWctx.close()
        psT = ctx.enter_context(tc.tile_pool(name="psT", bufs=2, space="PSUM"))
        psO = ctx.enter_context(tc.tile_pool(name="psO", bufs=1, space="PSUM"))
        accs = [psO.tile([OUT, 512], F32, tag=f"acc{ch}", name=f"acc{ch}")
                for ch in range(6)]
        out6 = sb.tile([OUT, C], F32)  # SBUF accumulator for pixels 3072:3200

        # xr2 viewed as overlapping runs: row i = elements [i*C, i*C + len)
        xr_pair = bass.AP(tensor=d_xr2.tensor, offset=0,
                          ap=[[C, XR_ROWS - 1], [1, 2 * C]])
        xr_quad = bass.AP(tensor=d_xr2.tensor, offset=0,
                          ap=[[C, XR_ROWS - 3], [1, 4 * C]])
        for tap in range(NTAP):
            g2 = gpool.tile([C, NTILE, 2 * C], BF16, tag="g2", name="g2")
            g4 = gpool.tile([C, 4, 4 * C], BF16, tag="g4", name="g4")
            # edge tiles 0,23,24: 4-corner quad, 1 desc/(pix,tap)
            nc.gpsimd.dma_gather(
                out_ap=g4[:, 0:3, :], in_ap=xr_quad,
                idxs_ap=widx_e[:, tap * 24: tap * 24 + 24],
                num_idxs=384, num_idxs_reg=384,
                elem_size=4 * C, elem_step=C)
            # interior tiles 1..22: anti-diagonal pair, 1 desc/(pix,tap);
            # chunked to fit the default 1024-entry SWDGE ring
            widx_h = widx_iA if tap < NTA else widx_iB
            tap_h = tap if tap < NTA else tap - NTA
            for j0, nj in ((0, 1024), (1024, 1024), (2048, 768)):
                c0 = tap_h * 176 + j0 // 16
                nc.gpsimd.dma_gather(
                    out_ap=g2[:, INT_T0 + j0 // 128: INT_T0 + (j0 + nj) // 128, :],
                    in_ap=xr_pair,
                    idxs_ap=widx_h[:, c0: c0 + nj // 16],
                    num_idxs=nj, num_idxs_reg=nj,
                    elem_size=2 * C, elem_step=C)

            rhs = rpool.tile([C, NPAD], BF16, tag="rhs")
            for tq in range(7):  # quads of pixel-tiles
                ntq = 4 if tq < 6 else 1
                pst = psT.tile([C, 512], F32, tag="pstr")
                pstb = pst[:].bitcast(BF16)
                for k in range(ntq):
                    t = tq * 4 + k
                    wcol = slice(t * NTAP + tap, t * NTAP + tap + 1)
                    xo = xpool.tile([C, C], BF16, tag="xo")
                    # last tap's blends run on the (then idle) Pool engine
                    # to relieve the DVE-bound tail; all gather launches
                    # have already been issued by then.
                    ve = nc.gpsimd if tap >= POOL_TAP else nc.vector
                    if t in EDGE_T:
                        ei = 0 if t == 0 else t - 22
                        ve.tensor_scalar(
                            xo[:], g4[:, ei, 0:C], wltb[:, wcol], None,
                            op0=ALU.mult)
                        ve.scalar_tensor_tensor(
                            xo[:], g4[:, ei, C:2 * C], wrtb[:, wcol], xo[:],
                            op0=ALU.mult, op1=ALU.add)
                        ve.scalar_tensor_tensor(
                            xo[:], g4[:, ei, 2 * C:3 * C], wlbb[:, wcol], xo[:],
                            op0=ALU.mult, op1=ALU.add)
                        ve.scalar_tensor_tensor(
                            xo[:], g4[:, ei, 3 * C:4 * C], wrbb[:, wcol], xo[:],
                            op0=ALU.mult, op1=ALU.add)
                    else:
                        ve.tensor_scalar(
                            xo[:], g2[:, t, 0:C], s0[:, wcol], None,
                            op0=ALU.mult)
                        ve.scalar_tensor_tensor(
                            xo[:], g2[:, t, C:2 * C], s1[:, wcol], xo[:],
                            op0=ALU.mult, op1=ALU.add)
                    nc.tensor.transpose(pstb[:, k * C:(k + 1) * C], xo[:],
                                        identb[:])
                nc.scalar.activation(rhs[:, tq * 512: tq * 512 + ntq * C],
                                     pstb[:, :ntq * C], ACTF.Copy)

            for ch in range(6):
                nc.tensor.matmul(
                    accs[ch][:],
                    convw[:, tap * OUT:(tap + 1) * OUT],
                    rhs[:, 512 * ch: 512 * ch + 512],
                    start=(tap == 0), stop=(tap == NTAP - 1))
            ps6 = psT.tile([C, 512], F32, tag="pstr")
            nc.tensor.matmul(ps6[:, :C],
                             convw[:, tap * OUT:(tap + 1) * OUT],
                             rhs[:, 3072:3200],
                             start=True, stop=True)
            if tap == 0:
                nc.vector.tensor_copy(out=out6[:], in_=ps6[:, :C])
            else:
                nc.vector.tensor_tensor(out6[:], out6[:], ps6[:, :C],
                                        op=ALU.add)

        # ---- F: output ----
        for ch in range(6):
            ob = opool.tile([OUT, 512], F32, tag="ob")
            nc.scalar.activation(ob[:], accs[ch][:], ACTF.Copy)
            nc.sync.dma_start(d_out[:, 512 * ch:512 * ch + 512], ob[:])
        nc.sync.dma_start(d_out[:, 3072:3200], out6[:])


# ---------------- host-side input prep ----------------

def prep_core_inputs(xb, offset_w, offset_b, conv_w):
    """Build the per-core in_map from one batch image [C, H, W] + weights."""
    f32 = np.float32
    xb = np.asarray(xb, f32)
    xp = np.pad(xb, ((0, 0), (1, 1), (1, 1)))                   # [C, 58, 58]
    xcp = np.ascontiguousarray(xp.reshape(C, HP * HP))
    xpm = np.ascontiguousarray(xp.transpose(1, 2, 0).reshape(HP * HP, C))
    xr60 = np.pad(xp, ((0, 0), (1, 1), (1, 1)), mode="edge")    # [C, 60, 60]
    xr60 = xr60.transpose(1, 2, 0)                              # [60, 60, C]
    # interleaved row pairs: xr2[2*(y*60+x)] = xr60[y,x]; [.. +1] = xr60[y+1,x]
    xr2 = np.stack([xr60[:-1], xr60[1:]], axis=2)               # [59, 60, 2, C]
    xr2 = np.ascontiguousarray(xr2.reshape(XR_ROWS, C)).astype(ml_dtypes.bfloat16)

    offw = np.empty((C, NTAP * 18), f32)
    convw = np.empty((C, NTAP * OUT), f32)
    for tap in range(NTAP):
        ky, kx = tap // 3, tap % 3
        offw[:, tap * 18:(tap + 1) * 18] = np.asarray(offset_w, f32)[:, :, ky, kx].T
        convw[:, tap * OUT:(tap + 1) * OUT] = np.asarray(conv_w, f32)[:, :, ky, kx].T
    offb = np.asarray(offset_b, f32).reshape(18, 1)

    # base grid [128, 25*18]: partition p, col t*18+k -> pixel t*128+p (col-major)
    r = np.arange(-1, 2, dtype=f32)
    py_n, px_n = np.meshgrid(r, r, indexing="ij")
    pny, pnx = py_n.ravel(), px_n.ravel()
    gy = np.arange(1, 57, dtype=f32)
    p0y, p0x = np.meshgrid(gy, gy, indexing="ij")
    p0yc, p0xc = p0y.T.ravel(), p0x.T.ravel()      # col-major pixels
    base = np.empty((NPAD, 18), f32)
    base[:NPIX, :NTAP] = p0yc[:, None] + pny[None, :]
    base[:NPIX, NTAP:] = p0xc[:, None] + pnx[None, :]
    base[NPIX:, :NTAP] = 28.0 + pny[None, :]
    base[NPIX:, NTAP:] = 28.0 + pnx[None, :]
    base = np.ascontiguousarray(
        base.reshape(NTILE, C, 18).transpose(1, 0, 2).reshape(C, NTILE * 18))

    tri = np.triu(np.ones((C, C), f32), 1)  # tri[p, m] = 1 iff p < m
    pixid1 = np.ascontiguousarray(
        (np.arange(NTILE)[None, :] * C + np.arange(C)[:, None] + 1)
        .astype(f32))
    return {"xcp": xcp, "xpm": xpm, "xr2": xr2, "offw": offw, "offb": offb,
            "convw": convw.astype(ml_dtypes.bfloat16), "base": base,
            "tri": tri, "pixid1": pixid1}


def postprocess(out_np):
    """[OUT, 3200] col-major -> [OUT, 56, 56]."""
    o = out_np[:, :NPIX].reshape(OUT, W, H).transpose(0, 2, 1)
    return np.ascontiguousarray(o)


# ---------------- entry point ----------------

N_CORES = 8
_cache = {}


def _build():
    if "nc" in _cache:
        return _cache["nc"]
    nc = bacc.Bacc("TRN2", target_bir_lowering=False, debug=False,
                   enable_asserts=True, num_devices=N_CORES)
    build_kernel(nc)
    nc.compile()
    nc.m = get_hw_module(nc.m)
    _cache["nc"] = nc
    return nc


def kernel(x, offset_w, offset_b, conv_w):
    x = np.asarray(x, np.float32)
    assert x.shape == (N_CORES, C, H, W), x.shape
    nc = _build()
    in_maps = [prep_core_inputs(x[b], offset_w, offset_b, conv_w)
               for b in range(N_CORES)]
    res = run_bass_kernel_spmd(nc, in_maps, core_ids=list(range(N_CORES)))
    outs = [postprocess(res.results[b]["out"]) for b in range(N_CORES)]
    return np.stack(outs).astype(np.float32)



# revision 37
# speedup vs baseline: 1.3420x; 1.0454x over previous
"""Deformable-conv kernel for Trainium2: 8-core data-parallel over batch.

kernel(x, offset_w, offset_b, conv_w) -> [8, 128, 56, 56] float32.
Each NeuronCore processes one batch image:
  offset conv in true-F32 PE matmuls (the reference sampler is discontinuous
  at integer x-coords, so offsets need ~1e-7 accuracy to reproduce its
  floor/trunc decisions) -> pixel-partitioned offsets (PE transpose)
  -> index/bilinear-weight math (DVE) -> bf16 indirect-DMA gathers from a
  row-pair-interleaved padded map (interior pixels: one 512B descriptor per
  (pixel, tap) fetching the anti-diagonal [bot-left, top-right] corner pair,
  index shifted by the exact-integer-hit mask; edge tiles: one 1KB 4-corner
  descriptor) -> 2-term (interior) / 4-term (edge) blend (DVE) -> bf16 PE
  transpose -> 3x3/stride-3 conv as 9 accumulating bf16 matmuls (PSUM).
"""
import sys
for _p in ("/opt/trn_rl_repo", "/root/.axon_site/_ro/trn_rl_repo"):
    if _p not in sys.path:
        sys.path.append(_p)

from contextlib import ExitStack

import numpy as np
import ml_dtypes

import concourse.bass as bass
import concourse.bacc as bacc
import concourse.mybir as mybir
import concourse.tile as tile
from concourse.masks import make_identity
from concourse.bass_utils import run_bass_kernel_spmd
from concourse.bass_interp import get_hw_module

F32 = mybir.dt.float32
BF16 = mybir.dt.bfloat16
I32 = mybir.dt.int32
I16 = mybir.dt.int16
ALU = mybir.AluOpType
ACTF = mybir.ActivationFunctionType

DEBUG_FIX = False
USE_FIXUP = False
USE_F32R = False
FIXLEVEL = 5  # bisect knob: 1=flag+zero+dram rt, 2=+rank/table, 3=+patch gather,
              # 4=+precise conv, 5=full (delta scatter-add)
POOL_TAP = 9  # taps >= this run blends on GPSIMD instead of DVE (9 = never:
              # measured slower at 7/8 — GPSIMD overhead + gather-launch
              # serialization outweigh the DVE relief)
H = W = 56
HP = 58
NPIX = H * W          # 3136
NPAD = 3200           # padded pixel count (25 tiles of 128)
NTILE = 25
NTAP = 9
C = 128
OUT = 128
XR_ROWS = 3540 * 2    # interleaved row-pair map: entry e -> rows 2e, 2e+1
# edge tiles: pixel cols j<=2 or j>=52 live here (clip/trunc can fire in x)
EDGE_T = (0, 23, 24)
INT_T0, INT_T1 = 1, 23  # interior tiles [1, 23)


def build_kernel(nc):
    d = {
        "xcp": nc.dram_tensor("xcp", [C, HP * HP], F32, kind="ExternalInput").ap(),
        "xpm": nc.dram_tensor("xpm", [HP * HP, C], F32, kind="ExternalInput").ap(),
        "xr2": nc.dram_tensor("xr2", [XR_ROWS, C], BF16, kind="ExternalInput").ap(),
        "offw": nc.dram_tensor("offw", [C, NTAP * 18], F32, kind="ExternalInput").ap(),
        "offb": nc.dram_tensor("offb", [18, 1], F32, kind="ExternalInput").ap(),
        "convw": nc.dram_tensor("convw", [C, NTAP * OUT], BF16, kind="ExternalInput").ap(),
        "base": nc.dram_tensor("base", [C, NTILE * 18], F32, kind="ExternalInput").ap(),
        "tri": nc.dram_tensor("tri", [C, C], F32, kind="ExternalInput").ap(),
        "pixid1": nc.dram_tensor("pixid1", [C, NTILE], F32, kind="ExternalInput").ap(),
        "out": nc.dram_tensor("out", [OUT, NPAD], F32, kind="ExternalOutput").ap(),
    }
    if DEBUG_FIX:
        for nm, shp, dt in (("dbg_offTm", [C, NTILE * 18], F32),
                            ("dbg_offT2", [C, NTILE * 18], F32),
                            ("dbg_fl", [C, NTILE], F32),
                            ("dbg_rank", [C, NTILE], F32),
                            ("dbg_tb", [16, 16], F32),
                            ("dbg_delta", [C, 2 * 18], F32),
                            ("dbg_prec", [18, 256], F32),
                            ("dbg_gk", [C, 3 * 2 * 3 * C], F32)):
            d[nm] = nc.dram_tensor(nm, shp, dt, kind="ExternalOutput").ap()
    with tile.TileContext(nc) as tc:
        emit(tc, d)
    return nc


def emit(tc, d):
    d_xcp, d_xpm, d_xr2 = d["xcp"], d["xpm"], d["xr2"]
    d_offw, d_offb, d_convw = d["offw"], d["offb"], d["convw"]
    d_base, d_tri, d_pixid1, d_out = d["base"], d["tri"], d["pixid1"], d["out"]
    nc = tc.nc
    F32R = mybir.dt.float32r
    ctx = ExitStack()
    with ctx:
        consts = ctx.enter_context(tc.tile_pool(name="consts", bufs=1))
        sb = ctx.enter_context(tc.tile_pool(name="sb", bufs=1))
        gpool = ctx.enter_context(tc.tile_pool(name="gpool", bufs=3))
        xpool = ctx.enter_context(tc.tile_pool(name="xpool", bufs=8))
        rpool = ctx.enter_context(tc.tile_pool(name="rpool", bufs=2))
        opool = ctx.enter_context(tc.tile_pool(name="opool", bufs=2))
        dpool = ctx.enter_context(tc.tile_pool(name="dpool", bufs=1, space="DRAM"))
        psWctx = ExitStack()
        psW = psWctx.enter_context(tc.tile_pool(name="psW", bufs=1,
                                                space="PSUM"))
        psBctx = ExitStack()
        psB = psBctx.enter_context(tc.tile_pool(name="psB", bufs=1, space="PSUM"))

        # ---- A: loads ----
        xcpr = None
        if USE_FIXUP or USE_F32R:
            xcpr = consts.tile([C, HP * HP], F32R)
            nc.gpsimd.dma_start(xcpr[:], d_xcp[:])
        xcpf = consts.tile([C, HP * HP], F32)
        nc.sync.dma_start(xcpf[:], d_xcp[:])
        offw = consts.tile([C, NTAP * 18], F32)
        nc.sync.dma_start(offw[:], d_offw[:])
        offwr = None
        if USE_FIXUP or USE_F32R:
            offwr = consts.tile([C, NTAP * 18], F32R)
            nc.gpsimd.dma_start(offwr[:], d_offw[:])
        convw = consts.tile([C, NTAP * OUT], BF16)
        nc.sync.dma_start(convw[:], d_convw[:])
        offb = consts.tile([18, 1], F32)
        nc.sync.dma_start(offb[:], d_offb[:])
        base = consts.tile([C, NTILE * 18], F32)
        nc.sync.dma_start(base[:], d_base[:])
        if USE_FIXUP:
            tri = consts.tile([C, C], F32)
            nc.sync.dma_start(tri[:], d_tri[:])
            pixid1 = consts.tile([C, NTILE], F32)
            nc.sync.dma_start(pixid1[:], d_pixid1[:])
        ident = consts.tile([C, C], F32)
        make_identity(nc, ident[:])
        identb = consts.tile([C, C], BF16)
        nc.vector.tensor_copy(out=identb[:], in_=ident[:])

        # PE p-state warmup: ~70 cheap matmuls fill the PE queue for >3us of
        # busy time (hidden under the input DMA loads), so phase B's matmuls
        # dispatch with the tensor clock fully ramped (cost-model p-state).
        wz = consts.tile([C, 64], BF16)
        nc.vector.memset(wz[:], 0.0)
        psw = psW.tile([1, 64], F32, tag="psw", name="psw")
        for _ in range(70):
            nc.tensor.matmul(psw[:], identb[:, 0:1], wz[:],
                             start=True, stop=True)

        # ---- B: offset conv, fast F32R pass (fixed up below for pixels whose
        # x-offset lands near an integer, where the reference's trunc/floor
        # decisions are discontinuous) ----
        # col-major output pixels: chunk c covers j in [8c, 8c+8), all i.
        off_sb = sb.tile([18, NPAD], F32)
        xcp3 = (xcpr if (USE_FIXUP or USE_F32R) else xcpf)[:].rearrange(
            "p (y x) -> p y x", y=HP)
        pss = [psB.tile([18, 448], F32, tag=f"psB{ch}", name=f"psB{ch}")
               for ch in range(7)]
        for tap in range(NTAP):
            ky, kx = tap // 3, tap % 3
            for ch in range(7):
                rhs = xcp3[:, ky:ky + 56, kx + 8 * ch: kx + 8 * ch + 8] \
                    .transpose([0, 2, 1])
                lhsw = offwr if (USE_FIXUP or USE_F32R) else offw
                nc.tensor.matmul(
                    pss[ch][:], lhsw[:, tap * 18:(tap + 1) * 18], rhs,
                    start=(tap == 0), stop=(tap == NTAP - 1))
        for ch in range(7):
            nc.scalar.activation(off_sb[:, 448 * ch:448 * (ch + 1)], pss[ch][:],
                                 ACTF.Identity, bias=offb[:, :1], scale=1.0)
        # pad pixels: 0.5 keeps them far from the near-integer flag band
        nc.vector.memset(off_sb[:, NPIX:], 0.5)

        # ---- C: transpose offsets to pixel-partitioned ----
        psBctx.close()
        psCctx = ExitStack()
        psC = psCctx.enter_context(tc.tile_pool(name="psC", bufs=2, space="PSUM"))
        offT = sb.tile([C, NTILE * 18], F32)
        for t in range(NTILE):
            pst = psC.tile([C, 18], F32, tag="psC")
            nc.tensor.transpose(pst[:], off_sb[:, t * C:(t + 1) * C],
                                ident[:18, :18])
            nc.scalar.activation(offT[:, t * 18:(t + 1) * 18], pst[:],
                                 ACTF.Copy)

        # ---- B2: precise fixup of near-integer x-offsets ----
        # Flag pixels with any x-offset within TH of an integer, zero their
        # x-offsets, round-trip offsets through DRAM (64-f32-padded pixel
        # rows), recompute flagged pixels' offsets with true-F32 matmuls on
        # gathered patches, and scatter-ADD them into the zeroed rows.
        def Y(ap):  # y-axis slice of [128, 25*18] -> [128, 25, 9]
            return ap[:].rearrange("p (t k) -> p t k", k=18)[:, :, 0:9]

        def X(ap):
            return ap[:].rearrange("p (t k) -> p t k", k=18)[:, :, 9:18]

        def V9(ap):  # [128, 25*9] -> [128, 25, 9]
            return ap[:].rearrange("p (t k) -> p t k", k=9)

        if USE_FIXUP:
            # Both x- AND y-offsets near an integer make the reference's
            # corner/trunc decisions discontinuous (only the anti-diagonal
            # corner pair survives generically, and which rows it sits on
            # flips at every y-integer crossing), so flag BOTH halves and
            # recompute flagged pixels' full 18-offset vector exactly.
            TH = 1e-3
            NSLOT = 256  # flagged-pixel capacity (E[flags] ~ 134/image)
            psF = psCctx.enter_context(tc.tile_pool(name="psF", bufs=1, space="PSUM"))
            d_offd = dpool.tile([4096, 64], F32, name="d_offd")
            d_ftab = dpool.tile([512, 64], F32, name="d_ftab")

            def Y(ap):  # y-axis slice of [128, 25*18] -> [128, 25, 9]
                return ap[:].rearrange("p (t k) -> p t k", k=18)[:, :, 0:9]

            def X(ap):
                return ap[:].rearrange("p (t k) -> p t k", k=18)[:, :, 9:18]

            def V18(ap):  # [128, 25*18] -> [128, 25, 18]
                return ap[:].rearrange("p (t k) -> p t k", k=18)

            fr = sb.tile([C, NTILE * 18], F32, tag="fxfr")
            fri = sb.tile([C, NTILE * 18], I32, tag="fxfri")
            fl2 = sb.tile([C, NTILE * 18], F32, tag="fxfl2")
            fl = sb.tile([C, NTILE], F32)
            rank = sb.tile([C, NTILE], F32)
            svec = sb.tile([C, 1], F32, tag="fxs")
            pbase = sb.tile([C, 1], F32)
            # fr = frac(off) via exact floor; near-integer iff fr<TH or fr>1-TH
            nc.vector.tensor_copy(out=V18(fri), in_=V18(offT))
            nc.vector.tensor_copy(out=V18(fr), in_=V18(fri))
            nc.vector.tensor_tensor(V18(fl2), V18(offT), V18(fr), op=ALU.is_lt)
            nc.vector.tensor_tensor(V18(fr), V18(fr), V18(fl2), op=ALU.subtract)
            nc.vector.tensor_tensor(V18(fr), V18(offT), V18(fr), op=ALU.subtract)
            nc.vector.tensor_scalar(V18(fl2), V18(fr), TH, None, op0=ALU.is_lt)
            nc.vector.tensor_scalar(V18(fr), V18(fr), 1.0 - TH, None, op0=ALU.is_gt)
            nc.vector.tensor_tensor(V18(fl2), V18(fl2), V18(fr), op=ALU.max)
            nc.vector.tensor_reduce(
                out=fl[:].rearrange("p (t u) -> p t u", u=1),
                in_=V18(fl2), axis=mybir.AxisListType.X, op=ALU.max)
            # zero flagged pixels' offsets (all 18) in place
            flb = fl2  # reuse
            nc.vector.tensor_scalar(flb[:, :NTILE], fl[:], -1.0, 1.0,
                                    op0=ALU.mult, op1=ALU.add)
            flbv = flb[:, :NTILE].rearrange("p (t u) -> p t u", u=1)
            flbb = bass.AP(tensor=flbv.tensor, offset=flbv.offset,
                           ap=[list(flbv.ap[0]), list(flbv.ap[1]), [0, 18]])
            nc.vector.tensor_tensor(V18(offT), V18(offT), flbb, op=ALU.mult)
            # offsets -> DRAM pixel rows (row = t*128+p, 64-f32 stride)
            od_w = d_offd[:].rearrange("(t p) c -> p t c", p=C)[:, :NTILE, 0:18]
            nc.sync.dma_start(od_w, offT[:])
            # ranks: pbase[p] = sum of flags on partitions < p; + exclusive scan
            if FIXLEVEL >= 2:
                nc.vector.tensor_reduce(out=svec[:], in_=fl[:],
                                        axis=mybir.AxisListType.X, op=ALU.add)
                psL = psF.tile([C, 1], F32, tag="psL")
                nc.tensor.matmul(psL[:], tri[:], svec[:], start=True, stop=True)
                nc.scalar.activation(pbase[:], psL[:], ACTF.Copy)
                nc.vector.tensor_tensor_scan(rank[:], fl[:], fl[:], initial=0.0,
                                             op0=ALU.add, op1=ALU.max)
                nc.vector.tensor_scalar(rank[:], rank[:], pbase[:, :1], None,
                                        op0=ALU.add)
                nc.vector.tensor_tensor(rank[:], rank[:], fl[:], op=ALU.subtract)
                nc.vector.tensor_scalar(rank[:], rank[:], NSLOT - 1.0, None,
                                        op0=ALU.min)
                # unflagged pixels -> dump slot NSLOT: every live slot gets
                # exactly one add (concurrent adds to one address lose updates)
                nc.vector.tensor_scalar(rank[:], rank[:], -float(NSLOT), None,
                                        op0=ALU.add)
                nc.vector.tensor_tensor(rank[:], rank[:], fl[:], op=ALU.mult)
                nc.vector.tensor_scalar(rank[:], rank[:], float(NSLOT), None,
                                        op0=ALU.add)
                # scatter fl*(pixid+1) into the NSLOT-slot table at rank
                vtab = sb.tile([C, NTILE], F32, tag="fxv")
                nc.vector.tensor_tensor(vtab[:], fl[:], pixid1[:], op=ALU.mult)
                rank16 = sb.tile([C, NTILE], I16)
                nc.vector.tensor_copy(out=rank16[:], in_=rank[:])
                rwr = sb.tile([C, 200], I16)
                for k in range(8):
                    dstr = rwr[0:16, :].rearrange(
                        "p (t k) -> p t k", k=8)[:, :, k:k + 1]
                    nc.sync.dma_start(dstr, rank16[16 * k:16 * k + 16, :].rearrange(
                        "p (t u) -> p t u", u=1))
                for lo, hi in ((16, 32), (32, 64), (64, 128)):
                    nc.sync.dma_start(rwr[lo:hi, :], rwr[0:hi - lo, :])
                zt = sb.tile([C, 1], F32, tag="fxz")
                nc.vector.memset(zt[:], 0.0)
                ft_head = d_ftab[:, 0:1]
                nc.sync.dma_start(d_ftab[0:C, 0:1], zt[:])
                nc.sync.dma_start(d_ftab[C:2 * C, 0:1], zt[:])
                nc.gpsimd.dma_scatter_add(
                    out_ap=ft_head,
                    in_ap=vtab[:].rearrange("p (a u) -> p a u", u=1),
                    idxs_ap=rwr[:, :],
                    num_idxs=NTILE * C, num_idxs_reg=NTILE * C,
                    elem_size=1, elem_step=64)
            if FIXLEVEL >= 3:
                # read table (wrapped 16x16), derive patch-run + scatter indices
                tb = sb.tile([16, 16], F32)
                tb_src = d_ftab[0:NSLOT, :].rearrange(
                    "(c r) u -> r c u", r=16)[:, :, 0:1]
                nc.sync.dma_start(tb[:], tb_src)
                pixv = sb.tile([16, 16], F32, tag="fxp")
                jj = sb.tile([16, 16], F32, tag="fxj")
                ji = sb.tile([16, 16], I32, tag="fxji")
                sc16 = sb.tile([C, 16], I16)
                nc.vector.tensor_scalar(pixv[:], tb[:], 3200.0, -1.0,
                                        op0=ALU.min, op1=ALU.add)
                nc.vector.tensor_copy(out=sc16[0:16, :], in_=pixv[:])  # -1 pads
                for lo, hi in ((16, 32), (32, 64), (64, 128)):
                    nc.sync.dma_start(sc16[lo:hi, :], sc16[0:hi - lo, :])
                nc.vector.tensor_scalar(pixv[:], pixv[:], 0.0, None, op0=ALU.max)
                # i32 copy rounds-to-nearest; bias by -0.5+eps so round == floor
                nc.vector.tensor_scalar(jj[:], pixv[:], 1.0 / 56, 1e-4 - 0.5,
                                        op0=ALU.mult, op1=ALU.add)
                nc.vector.tensor_copy(out=ji[:], in_=jj[:])
                nc.vector.tensor_copy(out=jj[:], in_=ji[:])
                # rbase = 58*i + j = 58*pix - 3247*j  (i = pix - 56*j)
                nc.vector.tensor_scalar(jj[:], jj[:], -3247.0, None, op0=ALU.mult)
                nc.vector.tensor_scalar(pixv[:], pixv[:], 58.0, None, op0=ALU.mult)
                nc.vector.tensor_tensor(pixv[:], pixv[:], jj[:], op=ALU.add)
                pidxf = sb.tile([16, 48], F32, tag="fxpi")
                for ky in range(3):
                    nc.vector.tensor_scalar(pidxf[:, ky * 16:(ky + 1) * 16],
                                            pixv[:], 58.0 * ky, None, op0=ALU.add)
                pidx = sb.tile([C, 48], I16)
                nc.vector.tensor_copy(out=pidx[0:16, :], in_=pidxf[:])
                for lo, hi in ((16, 32), (32, 64), (64, 128)):
                    nc.sync.dma_start(pidx[lo:hi, :], pidx[0:hi - lo, :])
                # gather 3x3-row patches (3 one-row-triple runs per flagged pixel)
                gk = sb.tile([C, 3, 2, 3 * C], F32)
                xpm_runs = bass.AP(tensor=d_xpm.tensor, offset=0,
                                   ap=[[C, HP * HP - 2], [1, 3 * C]])
                for ky in range(3):
                    nc.gpsimd.dma_gather(
                        out_ap=gk[:, ky, :, :], in_ap=xpm_runs,
                        idxs_ap=pidx[:, ky * 16:(ky + 1) * 16],
                        num_idxs=NSLOT, num_idxs_reg=NSLOT,
                        elem_size=3 * C, elem_step=C)
            if FIXLEVEL >= 4:
                # transpose patches to channel-major, precise F32 conv, add bias
                patchf = sb.tile([C, NTAP * NSLOT], F32)
                for tap in range(NTAP):
                    ky, kx = tap // 3, tap % 3
                    for ch in range(2):
                        psK = psF.tile([C, C], F32, tag="psK")
                        nc.tensor.transpose(
                            psK[:], gk[:, ky, ch, kx * C:(kx + 1) * C], ident[:])
                        nc.scalar.activation(
                            patchf[:, tap * NSLOT + ch * C:
                                   tap * NSLOT + (ch + 1) * C],
                            psK[:], ACTF.Copy)
                psP = psF.tile([18, NSLOT], F32, tag="psP")
                for tap in range(NTAP):
                    nc.tensor.matmul(psP[:], offw[:, tap * 18:(tap + 1) * 18],
                                     patchf[:, tap * NSLOT:(tap + 1) * NSLOT],
                                     start=(tap == 0), stop=(tap == NTAP - 1))
                prec = sb.tile([18, NSLOT], F32)
                nc.scalar.activation(prec[:], psP[:], ACTF.Identity,
                                     bias=offb[:, :1], scale=1.0)
                delta = sb.tile([C, 2, 18], F32)
                for ch in range(2):
                    psQ = psF.tile([C, 18], F32, tag="psQ")
                    nc.tensor.transpose(psQ[:], prec[:, ch * C:(ch + 1) * C],
                                        ident[:18, :18])
                    nc.scalar.activation(delta[:, ch, :], psQ[:], ACTF.Copy)
            if FIXLEVEL >= 5:
                # scatter full 18-offset rows into the zeroed DRAM rows
                # (idx < 0 at the end ignored)
                od_x = d_offd[:, 0:18]
                nc.gpsimd.dma_scatter_add(
                    out_ap=od_x,
                    in_ap=delta[:, :, :],
                    idxs_ap=sc16[:, :], num_idxs=NSLOT, num_idxs_reg=NSLOT,
                    elem_size=18, elem_step=64)
            # corrected offsets back to SBUF
            offT2 = sb.tile([C, NTILE * 18], F32)
            nc.sync.dma_start(offT2[:], od_w)
            if DEBUG_FIX:
                nc.sync.dma_start(d["dbg_offTm"][:], offT[:])
                nc.sync.dma_start(d["dbg_offT2"][:], offT2[:])
                nc.sync.dma_start(d["dbg_fl"][:], fl[:])
                nc.sync.dma_start(d["dbg_rank"][:], rank[:])
                nc.sync.dma_start(d["dbg_tb"][:], tb[:])
                nc.sync.dma_start(d["dbg_delta"][:],
                                  delta[:].rearrange("p a b -> p (a b)"))
                nc.sync.dma_start(d["dbg_prec"][:], prec[:])
                nc.sync.dma_start(d["dbg_gk"][:],
                                  gk[:].rearrange("p a b c -> p (a b c)"))

        else:
            offT2 = offT

        # ---- D: index + weight math ----
        # layout [128, 25*18]: col (t*18 + k), k in 0..8 = y taps, 9..17 = x taps
        w_lt = sb.tile([C, NTILE * NTAP], F32)
        w_rb = sb.tile([C, NTILE * NTAP], F32)
        w_lb = sb.tile([C, NTILE * NTAP], F32)
        w_rt = sb.tile([C, NTILE * NTAP], F32)
        s0f = sb.tile([C, NTILE * NTAP], F32)
        s1f = sb.tile([C, NTILE * NTAP], F32)
        idxf = sb.tile([C, NTILE * NTAP], F32)

        tmp = sb.tile([C, NTILE * 18], F32, tag="dtmp")      # p
        q = sb.tile([C, NTILE * 18], F32, tag="dtmp2")       # q = floor(p)
        qlt = sb.tile([C, NTILE * 18], F32, tag="dtmp3")
        qrb = sb.tile([C, NTILE * 18], F32, tag="dtmp4")
        pc = sb.tile([C, NTILE * 18], F32, tag="dtmp5")
        gA = sb.tile([C, NTILE * 18], F32, tag="dtmp6")      # 1 - f
        hh = sb.tile([C, NTILE * 18], F32, tag="dtmp7")      # 1 - (qrb - pc)
        t0 = sb.tile([C, NTILE * NTAP], F32, tag="dtmp8")
        t1 = sb.tile([C, NTILE * NTAP], F32, tag="dtmp9")
        tt = sb.tile([C, NTILE * NTAP], F32, tag="dtmp10")

        # per-half chain: y-half runs on the fast offsets (untouched by the
        # fixup) and overlaps the fixup's DMA latency; x-half waits for offT2
        ti = sb.tile([C, NTILE * 18], I32, tag="dti")

        def halfchain(V, src):
            nc.vector.tensor_tensor(V(tmp), V(base), V(src), op=ALU.add)
            nc.vector.tensor_copy(out=V(ti), in_=V(tmp))
            nc.vector.tensor_copy(out=V(q), in_=V(ti))
            nc.vector.tensor_tensor(V(qlt), V(tmp), V(q), op=ALU.is_lt)
            nc.vector.tensor_tensor(V(q), V(q), V(qlt), op=ALU.subtract)
            nc.vector.tensor_scalar(V(qlt), V(q), 0.0, 57.0,
                                    op0=ALU.max, op1=ALU.min)
            nc.vector.tensor_scalar(V(qrb), V(q), 1.0, 0.0,
                                    op0=ALU.add, op1=ALU.max)
            nc.vector.tensor_scalar(V(qrb), V(qrb), 57.0, None, op0=ALU.min)
            nc.vector.tensor_scalar(V(pc), V(tmp), 0.0, 57.0,
                                    op0=ALU.max, op1=ALU.min)
            nc.vector.tensor_tensor(V(gA), V(pc), V(qlt), op=ALU.subtract)
            nc.vector.tensor_scalar(V(gA), V(gA), -1.0, 1.0,
                                    op0=ALU.mult, op1=ALU.add)
            nc.vector.tensor_tensor(V(hh), V(qrb), V(pc), op=ALU.subtract)
            nc.vector.tensor_scalar(V(hh), V(hh), -1.0, 1.0,
                                    op0=ALU.mult, op1=ALU.add)

        halfchain(Y, offT2)
        halfchain(X, offT2)
        # trunc factors (x axis): t0 = (gA_x >= 1), t1 = (hh_x >= 1)
        nc.vector.tensor_scalar(V9(t0), X(gA), 1.0, None, op0=ALU.is_ge)
        nc.vector.tensor_scalar(V9(t1), X(hh), 1.0, None, op0=ALU.is_ge)
        # weights
        nc.vector.tensor_tensor(V9(w_lt), Y(gA), V9(t0), op=ALU.mult)
        nc.vector.tensor_tensor(V9(w_rb), Y(hh), V9(t1), op=ALU.mult)
        nc.vector.tensor_tensor(V9(w_lb), Y(gA), X(hh), op=ALU.mult)
        nc.vector.tensor_tensor(V9(w_rt), Y(hh), X(gA), op=ALU.mult)
        # interior slot weights (anti-diagonal pair + exact-hit fold):
        #   s0 = w_lt + w_rt*(1-t0)   (slot0 = bl normally, lt at exact hit)
        #   s1 = w_lb + t0*(w_rt - w_lb)  (slot1 = tr normally, bl at hit)
        nc.vector.tensor_tensor(tt[:], t0[:], w_rt[:], op=ALU.mult)
        nc.vector.tensor_tensor(s0f[:], w_lt[:], w_rt[:], op=ALU.add)
        nc.vector.tensor_tensor(s0f[:], s0f[:], tt[:], op=ALU.subtract)
        nc.vector.tensor_tensor(s1f[:], w_rt[:], w_lb[:], op=ALU.subtract)
        nc.vector.tensor_tensor(s1f[:], s1f[:], t0[:], op=ALU.mult)
        nc.vector.tensor_tensor(s1f[:], s1f[:], w_lb[:], op=ALU.add)
        # clip doubling: when x clips (t1=1, which implies t0=1) both
        # coincident corners contribute, so both slot weights double
        nc.vector.tensor_scalar(tt[:], t1[:], 1.0, None, op0=ALU.add)
        nc.vector.tensor_tensor(s0f[:], s0f[:], tt[:], op=ALU.mult)
        nc.vector.tensor_tensor(s1f[:], s1f[:], tt[:], op=ALU.mult)
        s0, s1 = s0f, s1f  # scalar operands must stay f32
        wltb, wrbb, wlbb, wrtb = w_lt, w_rb, w_lb, w_rt
        # gather base index: s = clip(q, -1, 57); e = sy*60 + sx + 61
        # idx rows (C-units) of xr2: edge tiles 2e; interior 2e + 1 - t0
        nc.vector.tensor_scalar(q[:], q[:], -1.0, 57.0, op0=ALU.max, op1=ALU.min)
        nc.vector.tensor_scalar(V9(idxf), Y(q), 120.0, 122.0,
                                op0=ALU.mult, op1=ALU.add)
        nc.vector.tensor_tensor(V9(idxf), V9(idxf), X(q), op=ALU.add)
        nc.vector.tensor_tensor(V9(idxf), V9(idxf), X(q), op=ALU.add)
        # all tiles: idx += 1 - t0 (edge clip/hit cases collapse onto the
        # same anti-diagonal row-pair once slot weights carry the 1+t1
        # doubling, so the 4-corner edge path is unnecessary)
        nc.vector.tensor_scalar(V9(idxf), V9(idxf), 1.0, None, op0=ALU.add)
        nc.vector.tensor_tensor(V9(idxf), V9(idxf), V9(t0), op=ALU.subtract)
        # int16 idx in per-tap layout [edge0 | edge23 | edge24 | interior 1..22]
        # (8 wrapped cols per tile) via wrap DMAs:
        #   interior: widx[r, n*200 + 24 + 8(t-1) + k] = idx16[16k + r, t*9 + n]
        idx16 = sb.tile([C, NTILE * NTAP], I16)
        nc.vector.tensor_copy(out=idx16[:], in_=idxf[:])
        # (n, t)-major splits: interior col n*22 + (t-1); edge col n*3 + ei
        i3 = idx16[:].rearrange("p (t n) -> p t n", t=NTILE)
        # taps split into half A (0..3) and half B (4..8) with SEPARATE wrap
        # tiles, so half A's gathers launch while half B's wraps still run.
        # All 25 tiles (incl. former edge tiles) share the pair-gather path.
        NTA, NTB = 4, 5
        idx16iA = sb.tile([C, NTA * NTILE], I16)
        idx16iB = sb.tile([C, NTB * NTILE], I16)
        nc.vector.tensor_copy(
            out=idx16iA[:].rearrange("p (n s) -> p s n", s=NTILE),
            in_=i3[:, :, 0:NTA])
        nc.vector.tensor_copy(
            out=idx16iB[:].rearrange("p (n s) -> p s n", s=NTILE),
            in_=i3[:, :, NTA:NTAP])
        widx_iA = sb.tile([C, NTA * 200], I16)  # col = n*200 + 8*t + k
        widx_iB = sb.tile([C, NTB * 200], I16)
        for k in range(8):
            dsti = widx_iA[0:16, :].rearrange(
                "p (s k) -> p s k", k=8)[:, :, k:k + 1]
            nc.sync.dma_start(dsti, idx16iA[16 * k:16 * k + 16, :].rearrange(
                "p (s u) -> p s u", u=1))
        for lo, hi in ((16, 32), (32, 64), (64, 128)):
            nc.sync.dma_start(widx_iA[lo:hi, :], widx_iA[0:hi - lo, :])
        for k in range(8):
            dsti = widx_iB[0:16, :].rearrange(
                "p (s k) -> p s k", k=8)[:, :, k:k + 1]
            nc.sync.dma_start(dsti, idx16iB[16 * k:16 * k + 16, :].rearrange(
                "p (s u) -> p s u", u=1))
        for lo, hi in ((16, 32), (32, 64), (64, 128)):
            nc.sync.dma_start(widx_iB[lo:hi, :], widx_iB[0:hi - lo, :])

        # keep the PE clock ramped through the DVE-heavy phase D: a few dummy
        # matmuls anchored on D outputs execute interspersed with D.
        for anchor in (w_lt, w_rb, w_lb, w_rt, s0f, s1f, idxf):
            nc.tensor.matmul(psw[:, 0:56], ident[:, 0:1], anchor[:, 0:56],
                             start=True, stop=True)

        # ---- E: per-tap gather + blend + transpose + conv ----
        psCctx.close()  # release phase-C PSUM banks
        psWctx.close()
        psT = ctx.enter_context(tc.tile_pool(name="psT", bufs=2, space="PSUM"))
        psO = ctx.enter_context(tc.tile_pool(name="psO", bufs=1, space="PSUM"))
        accs = [psO.tile([OUT, 512], F32, tag=f"acc{ch}", name=f"acc{ch}")
                for ch in range(6)]
        out6 = sb.tile([OUT, C], F32)  # SBUF accumulator for pixels 3072:3200

        # xr2 viewed as overlapping runs: row i = elements [i*C, i*C + len)
        xr_pair = bass.AP(tensor=d_xr2.tensor, offset=0,
                          ap=[[C, XR_ROWS - 1], [1, 2 * C]])
        xr_quad = bass.AP(tensor=d_xr2.tensor, offset=0,
                          ap=[[C, XR_ROWS - 3], [1, 4 * C]])
        for tap in range(NTAP):
            g2 = gpool.tile([C, NTILE, 2 * C], BF16, tag="g2", name="g2")
            # all 25 tiles: anti-diagonal pair, 1 desc/(pix,tap);
            # chunked to fit the default 1024-entry SWDGE ring
            widx_h = widx_iA if tap < NTA else widx_iB
            tap_h = tap if tap < NTA else tap - NTA
            for j0, nj in ((0, 1024), (1024, 1024), (2048, 1024), (3072, 128)):
                c0 = tap_h * 200 + j0 // 16
                nc.gpsimd.dma_gather(
                    out_ap=g2[:, j0 // 128: (j0 + nj) // 128, :],
                    in_ap=xr_pair,
                    idxs_ap=widx_h[:, c0: c0 + nj // 16],
                    num_idxs=nj, num_idxs_reg=nj,
                    elem_size=2 * C, elem_step=C)

            rhs = rpool.tile([C, NPAD], BF16, tag="rhs")
            for tq in range(7):  # quads of pixel-tiles
                ntq = 4 if tq < 6 else 1
                pst = psT.tile([C, 512], F32, tag="pstr")
                pstb = pst[:].bitcast(BF16)
                for k in range(ntq):
                    t = tq * 4 + k
                    wcol = slice(t * NTAP + tap, t * NTAP + tap + 1)
                    xo = xpool.tile([C, C], BF16, tag="xo")
                    # last tap's blends run on the (then idle) Pool engine
                    # to relieve the DVE-bound tail; all gather launches
                    # have already been issued by then.
                    ve = nc.gpsimd if tap >= POOL_TAP else nc.vector
                    ve.tensor_scalar(
                        xo[:], g2[:, t, 0:C], s0[:, wcol], None,
                        op0=ALU.mult)
                    ve.scalar_tensor_tensor(
                        xo[:], g2[:, t, C:2 * C], s1[:, wcol], xo[:],
                        op0=ALU.mult, op1=ALU.add)
                    nc.tensor.transpose(pstb[:, k * C:(k + 1) * C], xo[:],
                                        identb[:])
                nc.scalar.activation(rhs[:, tq * 512: tq * 512 + ntq * C],
                                     pstb[:, :ntq * C], ACTF.Copy)

            for ch in range(6):
                nc.tensor.matmul(
                    accs[ch][:],
                    convw[:, tap * OUT:(tap + 1) * OUT],
                    rhs[:, 512 * ch: 512 * ch + 512],
                    start=(tap == 0), stop=(tap == NTAP - 1))
            ps6 = psT.tile([C, 512], F32, tag="pstr")
            nc.tensor.matmul(ps6[:, :C],
                             convw[:, tap * OUT:(tap + 1) * OUT],
                             rhs[:, 3072:3200],
                             start=True, stop=True)
            if tap == 0:
                nc.vector.tensor_copy(out=out6[:], in_=ps6[:, :C])
            else:
                nc.vector.tensor_tensor(out6[:], out6[:], ps6[:, :C],
                                        op=ALU.add)

        # ---- F: output ----
        for ch in range(6):
            ob = opool.tile([OUT, 512], F32, tag="ob")
            nc.scalar.activation(ob[:], accs[ch][:], ACTF.Copy)
            nc.sync.dma_start(d_out[:, 512 * ch:512 * ch + 512], ob[:])
        nc.sync.dma_start(d_out[:, 3072:3200], out6[:])


# ---------------- host-side input prep ----------------

def prep_core_inputs(xb, offset_w, offset_b, conv_w):
    """Build the per-core in_map from one batch image [C, H, W] + weights."""
    f32 = np.float32
    xb = np.asarray(xb, f32)
    xp = np.pad(xb, ((0, 0), (1, 1), (1, 1)))                   # [C, 58, 58]
    xcp = np.ascontiguousarray(xp.reshape(C, HP * HP))
    xpm = np.ascontiguousarray(xp.transpose(1, 2, 0).reshape(HP * HP, C))
    xr60 = np.pad(xp, ((0, 0), (1, 1), (1, 1)), mode="edge")    # [C, 60, 60]
    xr60 = xr60.transpose(1, 2, 0)                              # [60, 60, C]
    # interleaved row pairs: xr2[2*(y*60+x)] = xr60[y,x]; [.. +1] = xr60[y+1,x]
    xr2 = np.stack([xr60[:-1], xr60[1:]], axis=2)               # [59, 60, 2, C]
    xr2 = np.ascontiguousarray(xr2.reshape(XR_ROWS, C)).astype(ml_dtypes.bfloat16)

    offw = np.empty((C, NTAP * 18), f32)
    convw = np.empty((C, NTAP * OUT), f32)
    for tap in range(NTAP):
        ky, kx = tap // 3, tap % 3
        offw[:, tap * 18:(tap + 1) * 18] = np.asarray(offset_w, f32)[:, :, ky, kx].T
        convw[:, tap * OUT:(tap + 1) * OUT] = np.asarray(conv_w, f32)[:, :, ky, kx].T
    offb = np.asarray(offset_b, f32).reshape(18, 1)

    # base grid [128, 25*18]: partition p, col t*18+k -> pixel t*128+p (col-major)
    r = np.arange(-1, 2, dtype=f32)
    py_n, px_n = np.meshgrid(r, r, indexing="ij")
    pny, pnx = py_n.ravel(), px_n.ravel()
    gy = np.arange(1, 57, dtype=f32)
    p0y, p0x = np.meshgrid(gy, gy, indexing="ij")
    p0yc, p0xc = p0y.T.ravel(), p0x.T.ravel()      # col-major pixels
    base = np.empty((NPAD, 18), f32)
    base[:NPIX, :NTAP] = p0yc[:, None] + pny[None, :]
    base[:NPIX, NTAP:] = p0xc[:, None] + pnx[None, :]
    base[NPIX:, :NTAP] = 28.0 + pny[None, :]
    base[NPIX:, NTAP:] = 28.0 + pnx[None, :]
    base = np.ascontiguousarray(
        base.reshape(NTILE, C, 18).transpose(1, 0, 2).reshape(C, NTILE * 18))

    tri = np.triu(np.ones((C, C), f32), 1)  # tri[p, m] = 1 iff p < m
    pixid1 = np.ascontiguousarray(
        (np.arange(NTILE)[None, :] * C + np.arange(C)[:, None] + 1)
        .astype(f32))
    return {"xcp": xcp, "xpm": xpm, "xr2": xr2, "offw": offw, "offb": offb,
            "convw": convw.astype(ml_dtypes.bfloat16), "base": base,
            "tri": tri, "pixid1": pixid1}


def postprocess(out_np):
    """[OUT, 3200] col-major -> [OUT, 56, 56]."""
    o = out_np[:, :NPIX].reshape(OUT, W, H).transpose(0, 2, 1)
    return np.ascontiguousarray(o)


# ---------------- entry point ----------------

N_CORES = 8
_cache = {}


def _build():
    if "nc" in _cache:
        return _cache["nc"]
    nc = bacc.Bacc("TRN2", target_bir_lowering=False, debug=False,
                   enable_asserts=True, num_devices=N_CORES)
    build_kernel(nc)
    nc.compile()
    nc.m = get_hw_module(nc.m)
    _cache["nc"] = nc
    return nc


def kernel(x, offset_w, offset_b, conv_w):
    x = np.asarray(x, np.float32)
    assert x.shape == (N_CORES, C, H, W), x.shape
    nc = _build()
    in_maps = [prep_core_inputs(x[b], offset_w, offset_b, conv_w)
               for b in range(N_CORES)]
    res = run_bass_kernel_spmd(nc, in_maps, core_ids=list(range(N_CORES)))
    outs = [postprocess(res.results[b]["out"]) for b in range(N_CORES)]
    return np.stack(outs).astype(np.float32)

